# revision 29
# baseline (speedup 1.0000x reference)
"""Trainium2 Bass kernel: batched ChebConv GNN with L1-distance adjacency.

Pipeline per sample (N=512 nodes, F=625 features):
  1. Sort nodes by attention (host). All pairs with |att_i-att_j| <= 0.05
     then lie within a rank band |i-j| <= w (w computed exactly on host).
  2. Banded pairwise L1 distances on device via the exact identity
     sum_f |a-b| = 2*sum_f max(a,b) - S_i - S_j  (S = row sums). The max
     runs on DVE in bf16 (2x DVE throughput; masks flip on only 0.08% of
     band pairs vs fp32); the feature reduction is a PE staircase matmul
     into one PSUM row per band offset.
  3. Threshold masks -> banded adjacency [w, N] -> PE-transpose to
     [128, w] tiles -> skewed-contiguous DMA into a dense upper-triangle
     DRAM matrix (168B runs, not 4B diagonal elements). Lower triangle
     rebuilt in SBUF from the readback via PE transposes (A symmetric).
  4. deg via PE column-sum matmuls; reciprocal row broadcast; ChebConv x2
     as bf16 PE matmuls in transposed layouts.
Data parallel over batch: 16 samples, 8 cores, 2 samples/core, the two
samples unit-interleaved so DVE/Pool/PE/Act all stay fed.
"""

import numpy as np
from contextlib import ExitStack

B, N = 16, 512
F, FH = 625, 937
FCH, NFCH = 125, 5  # feature chunks: 5 x 125 = 625
NCORES = 8
SPB = B // NCORES  # samples per core
DIST_THRESH, ATT_THRESH = 180.0, 0.05
DCH = 48  # band offsets per PSUM group

# FH row blocks (7x128 + 41)
FH_BLOCKS = [(o, min(128, FH - o)) for o in range(0, FH, 128)]

_prog_cache = {}


def _build_program(w, mm="bf16", reps=1, phase="all"):
    """Build the SPMD Bass program for band half-width w. Returns (nc, WROW).

    phase: which section the reps hardware loop wraps ("all", "phase1",
    "band", "cheb", "empty") — ablation instrument; result stays correct.
    """
    import os as _os
    import concourse.bass as bass
    import concourse.bacc as bacc
    import concourse.mybir as mybir
    import concourse.tile as tile
    from concourse.masks import make_identity

    dt = mybir.dt
    fp = dt.float32
    bf = dt.bfloat16
    AF = mybir.ActivationFunctionType
    OP = mybir.AluOpType
    AP = bass.AP

    DVE_BD = int(_os.environ.get("KERNEL_DVE_BD", "6"))
    POOL_N = int(_os.environ.get("KERNEL_POOL_N", "0"))  # offsets/chunk on Pool
    POOL_BD = int(_os.environ.get("KERNEL_POOL_BD", "6"))

    padw = ((w + 7) // 8) * 8
    WROW = N + padw  # padded row width for xt rows / a_scr cols

    nc = bacc.Bacc()
    xp_p = nc.declare_dram_parameter("xp", [SPB, N, F], bf, isOutput=False)
    xpt_p = nc.declare_dram_parameter("xpt", [SPB, F, WROW], bf, isOutput=False)
    attp_p = nc.declare_dram_parameter("attp", [SPB, WROW], fp, isOutput=False)
    w1_p = nc.declare_dram_parameter("w1", [2, F, FH], bf, isOutput=False)
    b1_p = nc.declare_dram_parameter("b1", [FH], fp, isOutput=False)
    w2_p = nc.declare_dram_parameter("w2", [2, FH, F], bf, isOutput=False)
    b2_p = nc.declare_dram_parameter("b2", [F], fp, isOutput=False)
    out_p = nc.declare_dram_parameter("outT", [SPB, F, N], fp, isOutput=True)
    ones_p = nc.declare_dram_parameter("c_ones", [128, 1], bf, isOutput=False)
    estep_p = nc.declare_dram_parameter("c_estep", [FCH, 95], bf, isOutput=False)
    zeros_p = nc.declare_dram_parameter("c_zeros", [128, WROW], bf, isOutput=False)

    # internal DRAM scratch: dense adjacency rows, one per sample slot
    a_scr = [nc.dram_tensor(f"a_scr{b}", [WROW * WROW], bf) for b in range(SPB)]

    with tile.TileContext(nc) as tc, ExitStack() as ctx:
        cst = ctx.enter_context(tc.tile_pool(name="cst", bufs=1))
        xtp = ctx.enter_context(tc.tile_pool(name="xtp", bufs=2))
        xpp = ctx.enter_context(tc.tile_pool(name="xpp", bufs=1))
        mxp = ctx.enter_context(tc.tile_pool(name="mxp", bufs=4))
        mpp = ctx.enter_context(tc.tile_pool(name="mpp", bufs=3))
        bnd = ctx.enter_context(tc.tile_pool(name="bnd", bufs=2))
        amp = ctx.enter_context(tc.tile_pool(name="amp", bufs=1))
        acp = ctx.enter_context(tc.tile_pool(name="acp", bufs=1))
        wsp = ctx.enter_context(tc.tile_pool(name="wsp", bufs=1))
        otp = ctx.enter_context(tc.tile_pool(name="otp", bufs=1))
        psb = ctx.enter_context(tc.tile_pool(name="psb", bufs=1, space="PSUM"))
        pst = ctx.enter_context(tc.tile_pool(name="pst", bufs=2, space="PSUM"))
        psp = ctx.enter_context(tc.tile_pool(name="psp", bufs=2, space="PSUM"))

        ones = cst.tile([128, 1], bf, tag="ones")
        nc.scalar.dma_start(ones[:], ones_p[:, :])
        identb = cst.tile([128, 128], bf, tag="identb")
        make_identity(nc, identb[:])
        # staircase selector: estep[:, 47-di : 47-di+dn] is a [FCH, dn]
        # matrix whose only nonzero column is column di (all ones) -> matmul
        # with it as lhsT reduces partitions into PSUM row di
        estep = cst.tile([FCH, 95], bf, tag="estep")
        nc.scalar.dma_start(estep[:], estep_p[:, :])

        # ---- setup (once): resident weights/biases, a_scr zero init
        w1t = [[wsp.tile([FCH, FH], bf, tag=f"w1t{k_}{c_}", name=f"w1t{k_}{c_}")
                for c_ in range(NFCH)] for k_ in range(2)]
        for k_ in range(2):
            for c_ in range(NFCH):
                nc.scalar.dma_start(w1t[k_][c_][:],
                                    w1_p[k_, c_ * FCH:(c_ + 1) * FCH, :])
        w2t = [[wsp.tile([128, F], bf, tag=f"w2t{k_}{j_}", name=f"w2t{k_}{j_}")
                for j_ in range(len(FH_BLOCKS))] for k_ in range(2)]
        for k_ in range(2):
            for j_, (ko, kp) in enumerate(FH_BLOCKS):
                nc.scalar.dma_start(w2t[k_][j_][:kp, :],
                                    w2_p[k_, ko:ko + kp, :])
        b1t = [wsp.tile([128, 1], fp, tag=f"b1t{j_}", name=f"b1t{j_}")
               for j_ in range(len(FH_BLOCKS))]
        for j_, (mo, mp_) in enumerate(FH_BLOCKS):
            nc.scalar.dma_start(b1t[j_][:mp_, :], b1_p[mo:mo + mp_])
        b2t = [wsp.tile([FCH, 1], fp, tag=f"b2t{m_}", name=f"b2t{m_}")
               for m_ in range(NFCH)]
        for m_ in range(NFCH):
            nc.scalar.dma_start(b2t[m_][:], b2_p[m_ * FCH:(m_ + 1) * FCH])
        for b in range(SPB):
            ad = a_scr[b]
            for t in range(5):
                lo = t * 128 * WROW
                n_el = min(128 * WROW, WROW * WROW - lo)
                if n_el <= 0:
                    break
                nc.sync.dma_start(AP(ad, lo, [[1, n_el]]),
                                  AP(zeros_p, 0, [[1, n_el]]))

        rep_cm = tc.For_i(0, reps, 1) if reps > 1 else None
        _lo = [False]

        def _open():
            if rep_cm is not None and not _lo[0]:
                rep_cm.__enter__()
                _lo[0] = True

        def _close():
            if _lo[0]:
                rep_cm.__exit__(None, None, None)
                _lo[0] = False

        if phase in ("all", "phase1"):
            _open()

        # ---------------- per-sample state ----------------
        xt_all = [None] * SPB
        xn_all = [None] * SPB
        srow_all = [None] * SPB
        at_all = [None] * SPB
        dinvB_all = [None] * SPB
        psM_all = [None] * SPB  # [DCH, N] bank triple-duty: psS row, band M, deg

        def gen_phase1(b):
            # single 3D DMA fills all five feature chunks side by side
            xtb = xtp.tile([FCH, NFCH * WROW], bf, tag=f"xt{b}", name=f"xt{b}")
            a = xtb[:]
            s = xpt_p[b, :, :]
            nc.scalar.dma_start(
                AP(a.tensor, a.offset,
                   [list(a.ap[0]), [WROW, NFCH], [1, WROW]]),
                AP(s.tensor, s.offset,
                   [[WROW, FCH], [FCH * WROW, NFCH], [1, WROW]]))
            xt = [xtb[:, c * WROW:(c + 1) * WROW] for c in range(NFCH)]
            xt_all[b] = xt
            # xn for cheb: one 3D DMA, issued early on SP queue
            xnb = xpp.tile([128, 4 * F], bf, tag=f"xn{b}", name=f"xn{b}")
            a = xnb[:]
            s = xp_p[b, :, :]
            nc.sync.dma_start(
                AP(a.tensor, a.offset, [list(a.ap[0]), [F, 4], [1, F]]),
                AP(s.tensor, s.offset, [[F, 128], [128 * F, 4], [1, F]]))
            xn = [xnb[:, t * F:(t + 1) * F] for t in range(4)]
            xn_all[b] = xn
            yield
            psMS = psb.tile([DCH, N], fp, tag=f"psM{b}", name=f"psM{b}")
            psM_all[b] = psMS
            for c in range(NFCH):
                nc.tensor.matmul(psMS[0:1, :], ones[:FCH, :], xt[c][:, :N],
                                 start=(c == 0), stop=(c == NFCH - 1))
            srow = bnd.tile([1, WROW], fp, tag=f"srow{b}", name=f"srow{b}")
            nc.gpsimd.memset(srow[:, N:], 0.0)
            nc.scalar.copy(srow[:, :N], psMS[0:1, :])
            srow_all[b] = srow
            yield

        def gen_band(b):
            ad = a_scr[b]
            xt, srow = xt_all[b], srow_all[b]
            psM = psM_all[b]
            d0 = 1
            while d0 <= w:
                dn = min(DCH, w - d0 + 1)
                # offsets [d0, d0+dn): tail POOL_N of them on Pool engine.
                # steps = (engine, offset-batch, chunk); pool steps merged
                # evenly among dve steps so the in-order PSUM consumption
                # chain lets both engines produce concurrently
                n_pool = min(POOL_N, max(0, dn - 1)) if POOL_N > 0 else 0
                n_dve = dn - n_pool
                dsteps, psteps = [], []
                o = 0
                while o < n_dve:
                    nb = min(DVE_BD, n_dve - o)
                    for c in range(NFCH):
                        dsteps.append(("dve", o, nb, c))
                    o += nb
                while o < dn:
                    nb = min(POOL_BD, dn - o)
                    for c in range(NFCH):
                        psteps.append(("pool", o, nb, c))
                    o += nb
                steps = []
                nd, np_ = len(dsteps), len(psteps)
                di_, pi_ = 0, 0
                while di_ < nd or pi_ < np_:
                    # keep pool's consumed fraction slightly ahead
                    if pi_ < np_ and (di_ >= nd or
                                      pi_ * nd <= di_ * np_):
                        steps.append(psteps[pi_])
                        pi_ += 1
                    else:
                        steps.append(dsteps[di_])
                        di_ += 1
                total_mm = dn * NFCH
                mm_done = 0
                for eng, db0, nb, c in steps:
                    tp = mxp if eng == "dve" else mpp
                    bd_ = DVE_BD if eng == "dve" else POOL_BD
                    mxb = tp.tile([FCH, bd_ * N], bf, tag=f"mx_{eng}",
                                  name=f"mx_{eng}")
                    base = xt[c][:, 0:N]
                    in0 = AP(base.tensor, base.offset,
                             [list(base.ap[0]), [0, nb], [1, N]])
                    in1 = AP(base.tensor, base.offset + d0 + db0,
                             [list(base.ap[0]), [1, nb], [1, N]])
                    e = nc.vector if eng == "dve" else nc.gpsimd
                    e.tensor_tensor(out=mxb[:, :nb * N], in0=in0, in1=in1,
                                    op=OP.max)
                    for j in range(nb):
                        di = db0 + j
                        mm_done += 1
                        nc.tensor.matmul(
                            psM[:dn, :],
                            estep[:, 47 - di:47 - di + dn],
                            mxb[:, j * N:(j + 1) * N],
                            start=(mm_done == 1),
                            stop=(mm_done == total_mm))
                    yield
                # epilogue: D = 2M - S_i - S_{i+d}; masks -> abnd
                sv = srow[:, :]
                sshift = bnd.tile([dn, N], fp, tag=f"sshift{b}", bufs=1,
                                  name="sshift")
                nc.sync.dma_start(
                    sshift[:],
                    AP(sv.tensor, sv.offset + d0,
                       [list(sv.ap[0]), [1, dn], [1, N]]))
                sb_t = bnd.tile([dn, N], fp, tag=f"sb{b}", bufs=1, name="sb_t")
                nc.sync.dma_start(
                    sb_t[:],
                    AP(sv.tensor, sv.offset,
                       [list(sv.ap[0]), [0, dn], [1, N]]))
                ashift = bnd.tile([dn, N], fp, tag=f"ashift{b}", bufs=1,
                                  name="ashift")
                nc.scalar.dma_start(ashift[:],
                                    AP(attp_p, b * WROW + d0, [[1, dn], [1, N]]))
                ab_t = bnd.tile([dn, N], fp, tag=f"ab{b}", bufs=1, name="ab_t")
                nc.scalar.dma_start(ab_t[:],
                                    AP(attp_p, b * WROW, [[0, dn], [1, N]]))
                yield
                nc.vector.scalar_tensor_tensor(
                    out=sb_t[:], in0=sb_t[:], scalar=DIST_THRESH, in1=sshift[:],
                    op0=OP.add, op1=OP.add)
                nc.vector.scalar_tensor_tensor(
                    out=sshift[:], in0=psM[:dn, :], scalar=2.0, in1=sb_t[:],
                    op0=OP.mult, op1=OP.is_le)
                nc.vector.tensor_sub(ashift[:], ashift[:], ab_t[:])
                nc.vector.tensor_scalar(ab_t[:], ashift[:], ATT_THRESH, None,
                                        op0=OP.is_le)
                abnd = bnd.tile([dn, N], bf, tag=f"abnd{b}", name="abnd")
                nc.vector.tensor_mul(abnd[:], sshift[:], ab_t[:])
                yield
                # transpose to [128, dn] blocks; skewed-contiguous scatter of
                # the upper triangle: A[i, i+d] for d in [d0, d0+dn) lands at
                # a_scr[i*(WROW+1) + d0 + f], contiguous runs of dn elems
                abT = bnd.tile([128, 4 * DCH], bf, tag=f"abT{b}",
                               name="abT", bufs=2)
                for t in range(4):
                    psTa = pst.tile([128, 128], bf, tag="pst",
                                    name="psTa")
                    nc.tensor.transpose(psTa[:, :dn],
                                        abnd[:, t * 128:(t + 1) * 128],
                                        identb[:dn, :dn])
                    nc.scalar.copy(abT[:, t * DCH:t * DCH + dn],
                                   psTa[:, :dn])
                    yield
                av = abT[:]
                nc.sync.dma_start(
                    AP(ad, d0, [[WROW + 1, 128],
                                [128 * (WROW + 1), 4], [1, dn]]),
                    AP(av.tensor, av.offset,
                       [list(av.ap[0]), [DCH, 4], [1, dn]]))
                yield
                d0 += dn

            # dense readback (upper + zeros elsewhere), rebuild lower
            # triangle from symmetric upper via PE transposes
            atb = amp.tile([128, 4 * N], bf, tag=f"at{b}", name=f"at{b}")
            a = atb[:]
            nc.sync.dma_start(
                AP(a.tensor, a.offset, [list(a.ap[0]), [N, 4], [1, N]]),
                AP(ad, 0, [[WROW, 128], [128 * WROW, 4], [1, N]]))
            at = [atb[:, t * N:(t + 1) * N] for t in range(4)]
            yield
            for t in range(4):
                # diagonal block: lower part = upper(t,t)^T; also add I
                psT2 = pst.tile([128, 128], bf, tag="pst", name="psT2")
                nc.tensor.transpose(psT2[:],
                                    at[t][:, t * 128:(t + 1) * 128],
                                    identb[:])
                nc.vector.tensor_tensor(
                    out=at[t][:, t * 128:(t + 1) * 128],
                    in0=at[t][:, t * 128:(t + 1) * 128],
                    in1=psT2[:], op=OP.add)
                nc.vector.tensor_tensor(
                    out=at[t][:, t * 128:(t + 1) * 128],
                    in0=at[t][:, t * 128:(t + 1) * 128],
                    in1=identb[:], op=OP.add)
                if t > 0:
                    # wedge: rows of block t, cols of block t-1
                    psT3 = pst.tile([128, 128], bf, tag="pst",
                                    name="psT3")
                    nc.tensor.transpose(psT3[:],
                                        at[t - 1][:, t * 128:(t + 1) * 128],
                                        identb[:])
                    nc.vector.tensor_tensor(
                        out=at[t][:, (t - 1) * 128:t * 128],
                        in0=at[t][:, (t - 1) * 128:t * 128],
                        in1=psT3[:], op=OP.add)
                yield
            # deg[j] = colsum (A+I); scale cols by 1/deg
            psM2 = psM_all[b]
            for t in range(4):
                nc.tensor.matmul(psM2[0:1, :], ones[:, :], at[t][:],
                                 start=(t == 0), stop=(t == 3))
            dinvR = bnd.tile([1, N], fp, tag=f"dinvR{b}", name=f"dinvR{b}")
            nc.vector.reciprocal(dinvR[:], psM2[0:1, :])
            dinvB = amp.tile([128, N], fp, tag=f"dinvB{b}", name=f"dinvB{b}")
            nc.gpsimd.partition_broadcast(dinvB[:], dinvR[:, :])
            dinvB_all[b] = dinvB
            at_all[b] = at
            yield

        def gen_cheb(b):
            xt, at, xn = xt_all[b], at_all[b], xn_all[b]
            dinvB = dinvB_all[b]
            zt = [acp.tile([FCH, N], bf, tag=f"zt{b}{m}", name=f"zt{b}{m}")
                  for m in range(NFCH)]
            for m in range(NFCH):
                psZ = psp.tile([FCH, N], fp, tag=f"mm{b}", name="psZ")
                for t in range(4):
                    nc.tensor.matmul(psZ[:], xn[t][:, m * FCH:(m + 1) * FCH],
                                     at[t][:], start=(t == 0), stop=(t == 3))
                nc.vector.tensor_mul(zt[m][:], psZ[:], dinvB[:FCH, :])
                yield

            ht = [acp.tile([128, N], bf, tag=f"ht{b}{k}", name=f"ht{b}{k}")
                  for k in range(len(FH_BLOCKS))]
            for k, (mo, mp_) in enumerate(FH_BLOCKS):
                psH = psp.tile([128, N], fp, tag=f"mm{b}", name="psH")
                for c in range(NFCH):
                    nc.tensor.matmul(psH[:mp_, :], w1t[0][c][:, mo:mo + mp_],
                                     xt[c][:, :N], start=(c == 0), stop=False)
                for c in range(NFCH):
                    nc.tensor.matmul(psH[:mp_, :], w1t[1][c][:, mo:mo + mp_],
                                     zt[c][:], start=False,
                                     stop=(c == NFCH - 1))
                nc.scalar.activation(ht[k][:mp_, :], psH[:mp_, :], AF.Relu,
                                     bias=b1t[k][:mp_, :], scale=1.0)
                yield

            qt = [acp.tile([128, N], bf, tag=f"qt{b}{k}", name=f"qt{b}{k}")
                  for k in range(len(FH_BLOCKS))]
            for k, (mo, mp_) in enumerate(FH_BLOCKS):
                psQ = psp.tile([128, N], fp, tag=f"mm{b}", name="psQ")
                for t in range(4):
                    psT = pst.tile([128, 128], bf, tag="pst", name="psT")
                    nc.tensor.transpose(
                        psT[:, :mp_],
                        ht[k][:mp_, t * 128:(t + 1) * 128],
                        identb[:mp_, :mp_])
                    hb = bnd.tile([128, 128], bf, tag=f"hb{b}", bufs=3,
                                  name="hb")
                    nc.scalar.copy(hb[:, :mp_], psT[:, :mp_])
                    nc.tensor.matmul(psQ[:mp_, :], hb[:, :mp_], at[t][:],
                                     start=(t == 0), stop=(t == 3))
                nc.vector.tensor_mul(qt[k][:mp_, :], psQ[:mp_, :],
                                     dinvB[:mp_, :])
                yield

            for m in range(NFCH):
                psO = psp.tile([FCH, N], fp, tag=f"mm{b}", name="psO")
                for k, (ko, kp) in enumerate(FH_BLOCKS):
                    nc.tensor.matmul(psO[:],
                                     w2t[0][k][:kp, m * FCH:(m + 1) * FCH],
                                     ht[k][:kp, :], start=(k == 0), stop=False)
                for k, (ko, kp) in enumerate(FH_BLOCKS):
                    nc.tensor.matmul(psO[:],
                                     w2t[1][k][:kp, m * FCH:(m + 1) * FCH],
                                     qt[k][:kp, :], start=False,
                                     stop=(k == len(FH_BLOCKS) - 1))
                ot = otp.tile([FCH, N], fp, tag=f"ot{b}", name="ot")
                nc.scalar.activation(ot[:], psO[:], AF.Relu, bias=b2t[m][:],
                                     scale=1.0)
                nc.sync.dma_start(out_p[b, m * FCH:(m + 1) * FCH, :], ot[:])
                yield

        def rr(*gens):
            gens = list(gens)
            while gens:
                g = gens.pop(0)
                if next(g, StopIteration) is not StopIteration:
                    gens.append(g)

        def stagger(g_a, g_b, ratio=2):
            # drive g_a `ratio` steps per g_b step until both exhausted
            done_a = done_b = False
            while not (done_a and done_b):
                for _ in range(ratio):
                    if not done_a:
                        done_a = next(g_a, StopIteration) is StopIteration
                if not done_b:
                    done_b = next(g_b, StopIteration) is StopIteration

        if phase == "all":
            rr(gen_phase1(0), gen_phase1(1))
            for _ in gen_band(0):
                pass
            stagger(gen_band(1), gen_cheb(0), ratio=2)
            for _ in gen_cheb(1):
                pass
        else:
            rr(gen_phase1(0), gen_phase1(1))
            if phase == "phase1":
                _close()
            if phase == "band":
                _open()
            rr(gen_band(0), gen_band(1))
            if phase == "band":
                _close()
            if phase == "cheb":
                _open()
            rr(gen_cheb(0), gen_cheb(1))
            if phase == "cheb":
                _close()
            if phase == "empty":
                _open()
                dummy = bnd.tile([1, 1], fp, tag="dummy", name="dummy")
                nc.scalar.copy(dummy[:], ones[:1, :].bitcast(fp))
                _close()
        _close()

    if not nc.is_finalized():
        nc.finalize()
    return nc, WROW


def _prepare(x4, attention):
    """Host prep: flatten, sort by attention, compute band width, pad."""
    X = np.ascontiguousarray(x4.reshape(B, N, F), dtype=np.float32)
    att = np.ascontiguousarray(attention[:, :, 0, 0], dtype=np.float32)
    perms = np.argsort(att, axis=1, kind="stable")
    attp = np.take_along_axis(att, perms, axis=1)
    a64 = attp.astype(np.float64)
    w = 1
    for bi in range(B):
        for d in range(1, N):
            if np.min(a64[bi, d:] - a64[bi, :-d]) <= ATT_THRESH + 1e-6:
                w = max(w, d)
            else:
                break  # windows only widen with d
    w = min(w, N - 1)
    Xp = np.take_along_axis(X, perms[:, :, None], axis=1)
    return Xp, attp, perms, w


def _make_runner(nc):
    """Compile the Bass program into a reusable 8-core sharded jax callable.

    Mirrors concourse.bass2jax.run_bass_via_pjrt's multi-core branch, but
    returns the compiled callable so repeated executions can be timed.
    """
    import jax
    from jax.sharding import Mesh, PartitionSpec
    from jax.experimental.shard_map import shard_map
    from concourse import bass2jax, mybir

    bass2jax.install_neuronx_cc_hook()

    in_names, out_names, out_avals, zero_outs = [], [], [], []
    partition_name = (nc.partition_id_tensor.name
                      if nc.partition_id_tensor else None)
    for alloc in nc.m.functions[0].allocations:
        if not isinstance(alloc, mybir.MemoryLocationSet):
            continue
        name = alloc.memorylocations[0].name
        if alloc.kind == "ExternalInput":
            if name != partition_name:
                in_names.append(name)
        elif alloc.kind == "ExternalOutput":
            shape = tuple(alloc.tensor_shape)
            dtype = mybir.dt.np(alloc.dtype)
            out_names.append(name)
            out_avals.append(jax.core.ShapedArray(shape, dtype))
            zero_outs.append(np.zeros(shape, dtype))
    n_params = len(in_names)
    n_outs = len(out_avals)
    in_names = in_names + out_names
    if partition_name is not None:
        in_names.append(partition_name)
    donate = tuple(range(n_params, n_params + n_outs))

    def _body(*args):
        operands = list(args)
        if partition_name is not None:
            operands.append(bass2jax.partition_id_tensor())
        outs = bass2jax._bass_exec_p.bind(
            *operands,
            out_avals=tuple(out_avals),
            in_names=tuple(in_names),
            out_names=tuple(out_names),
            lowering_input_output_aliases=(),
            sim_require_finite=True,
            sim_require_nnan=True,
            nc=nc,
        )
        return tuple(outs)

    devices = jax.devices()[:NCORES]
    mesh = Mesh(np.asarray(devices), ("core",))
    sharded = jax.jit(
        shard_map(_body, mesh=mesh,
                  in_specs=(PartitionSpec("core"),) * (n_params + n_outs),
                  out_specs=(PartitionSpec("core"),) * n_outs,
                  check_rep=False),
        donate_argnums=donate, keep_unused=True)

    param_order = in_names[:n_params]

    def run(in_maps):
        concat_in = [
            np.concatenate([np.asarray(in_maps[c][nm]) for c in range(NCORES)],
                           axis=0)
            for nm in param_order
        ]
        concat_zeros = [np.zeros((NCORES * z.shape[0], *z.shape[1:]), z.dtype)
                        for z in zero_outs]
        out_arrs = jax.block_until_ready(sharded(*concat_in, *concat_zeros))
        return [
            {nm: np.asarray(out_arrs[i]).reshape(NCORES, *out_avals[i].shape)[c]
             for i, nm in enumerate(out_names)}
            for c in range(NCORES)
        ]

    return {"run": run, "sharded": sharded, "param_order": param_order,
            "zero_outs": zero_outs, "out_names": out_names,
            "out_avals": out_avals, "mesh": mesh}


def _get_runner(w, reps=None):
    import os
    mm = os.environ.get("KERNEL_MM_DTYPE", "bf16")
    if reps is None:
        reps = int(os.environ.get("KERNEL_REPS", "1"))
    phase = os.environ.get("KERNEL_PHASE", "all")
    key = (w, mm, reps, phase,
           os.environ.get("KERNEL_DVE_BD", ""),
           os.environ.get("KERNEL_POOL_N", ""),
           os.environ.get("KERNEL_POOL_BD", ""))
    if key not in _prog_cache:
        nc, WROW = _build_program(w, mm=mm, reps=reps, phase=phase)
        _prog_cache[key] = (_make_runner(nc), WROW)
    return _prog_cache[key]


def kernel(x4, attention, W1, b1, W2, b2):
    import ml_dtypes
    bf16 = ml_dtypes.bfloat16
    Xp, attp, perms, w = _prepare(x4, attention)
    runner, WROW = _get_runner(w, reps=1)

    Xpb = Xp.astype(bf16)
    xpt = np.zeros((B, F, WROW), bf16)
    xpt[:, :, :N] = Xpb.transpose(0, 2, 1)
    attp_pad = np.full((B, WROW), 1e9, np.float32)
    attp_pad[:, :N] = attp

    W1b = np.ascontiguousarray(W1, dtype=np.float32).astype(bf16)
    W2b = np.ascontiguousarray(W2, dtype=np.float32).astype(bf16)
    b1 = np.ascontiguousarray(b1, dtype=np.float32)
    b2 = np.ascontiguousarray(b2, dtype=np.float32)

    c_ones = np.ones((128, 1), bf16)
    c_estep = np.zeros((FCH, 95), bf16)
    c_estep[:, 47] = 1.0
    c_zeros = np.zeros((128, WROW), bf16)

    in_maps = []
    for c in range(NCORES):
        sl = slice(c * SPB, (c + 1) * SPB)
        in_maps.append({
            "xp": np.ascontiguousarray(Xpb[sl]),
            "xpt": np.ascontiguousarray(xpt[sl]),
            "attp": np.ascontiguousarray(attp_pad[sl]),
            "w1": W1b, "b1": b1, "w2": W2b, "b2": b2,
            "c_ones": c_ones, "c_estep": c_estep, "c_zeros": c_zeros,
        })

    results = runner["run"](in_maps)
    globals()["last_in_maps"] = in_maps
    globals()["last_runner"] = runner

    inv = np.argsort(perms, axis=1)
    out = np.empty((B, N, F), np.float32)
    for c in range(NCORES):
        o = results[c]["outT"]  # [SPB, F, N]
        for s in range(SPB):
            bi = c * SPB + s
            out[bi] = o[s].T[inv[bi]]
    return out


# revision 31
# speedup vs baseline: 1.1330x; 1.1330x over previous
"""Trainium2 Bass kernel: batched ChebConv GNN with L1-distance adjacency.

Pipeline per sample (N=512 nodes, F=625 features):
  1. Sort nodes by attention (host). All pairs with |att_i-att_j| <= 0.05
     then lie within a rank band |i-j| <= w (w computed exactly on host).
  2. Banded pairwise L1 distances on device via the exact identity
     sum_f |a-b| = 2*sum_f max(a,b) - S_i - S_j  (S = row sums). The max
     runs on DVE in bf16 (2x DVE throughput; masks flip on only 0.08% of
     band pairs vs fp32); the feature reduction is a PE staircase matmul
     into one PSUM row per band offset.
  3. Threshold masks -> banded adjacency [w, N] -> PE-transpose to
     [128, w] tiles -> skewed-contiguous DMA into a dense upper-triangle
     DRAM matrix (168B runs, not 4B diagonal elements). Lower triangle
     rebuilt in SBUF from the readback via PE transposes (A symmetric).
  4. deg via PE column-sum matmuls; reciprocal row broadcast; ChebConv x2
     as bf16 PE matmuls in transposed layouts.
Data parallel over batch: 16 samples, 8 cores, 2 samples/core, the two
samples unit-interleaved so DVE/Pool/PE/Act all stay fed.
"""

import numpy as np
from contextlib import ExitStack

B, N = 16, 512
F, FH = 625, 937
FCH, NFCH = 125, 5  # feature chunks: 5 x 125 = 625
NCORES = 8
SPB = B // NCORES  # samples per core
DIST_THRESH, ATT_THRESH = 180.0, 0.05
DCH = 48  # band offsets per PSUM group

# FH row blocks (7x128 + 41)
FH_BLOCKS = [(o, min(128, FH - o)) for o in range(0, FH, 128)]

_prog_cache = {}


def _build_program(w, mm="bf16", reps=1, phase="all"):
    """Build the SPMD Bass program for band half-width w. Returns (nc, WROW).

    phase: which section the reps hardware loop wraps ("all", "phase1",
    "band", "cheb", "empty") — ablation instrument; result stays correct.
    """
    import os as _os
    import concourse.bass as bass
    import concourse.bacc as bacc
    import concourse.mybir as mybir
    import concourse.tile as tile
    from concourse.masks import make_identity

    dt = mybir.dt
    fp = dt.float32
    bf = dt.bfloat16
    AF = mybir.ActivationFunctionType
    OP = mybir.AluOpType
    AP = bass.AP

    DVE_BD = int(_os.environ.get("KERNEL_DVE_BD", "6"))
    POOL_N = int(_os.environ.get("KERNEL_POOL_N", "0"))  # offsets/chunk on Pool
    POOL_BD = int(_os.environ.get("KERNEL_POOL_BD", "6"))

    padw = ((w + 7) // 8) * 8
    WROW = N + padw  # padded row width for xt rows / a_scr cols

    nc = bacc.Bacc()
    xp_p = nc.declare_dram_parameter("xp", [SPB, N, F], bf, isOutput=False)
    xpt_p = nc.declare_dram_parameter("xpt", [SPB, F, WROW], bf, isOutput=False)
    attp_p = nc.declare_dram_parameter("attp", [SPB, WROW], fp, isOutput=False)
    w1_p = nc.declare_dram_parameter("w1", [2, F, FH], bf, isOutput=False)
    b1_p = nc.declare_dram_parameter("b1", [FH], fp, isOutput=False)
    w2_p = nc.declare_dram_parameter("w2", [2, FH, F], bf, isOutput=False)
    b2_p = nc.declare_dram_parameter("b2", [F], fp, isOutput=False)
    out_p = nc.declare_dram_parameter("outT", [SPB, F, N], fp, isOutput=True)
    ones_p = nc.declare_dram_parameter("c_ones", [128, 1], bf, isOutput=False)
    estep_p = nc.declare_dram_parameter("c_estep", [FCH, 95], bf, isOutput=False)
    zeros_p = nc.declare_dram_parameter("c_zeros", [128, WROW], bf, isOutput=False)

    # internal DRAM scratch: dense adjacency rows, one per sample slot
    a_scr = [nc.dram_tensor(f"a_scr{b}", [WROW * WROW], bf) for b in range(SPB)]

    with tile.TileContext(nc) as tc, ExitStack() as ctx:
        cst = ctx.enter_context(tc.tile_pool(name="cst", bufs=1))
        xtp = ctx.enter_context(tc.tile_pool(name="xtp", bufs=2))
        xpp = ctx.enter_context(tc.tile_pool(name="xpp", bufs=1))
        mxp = ctx.enter_context(tc.tile_pool(name="mxp", bufs=3))
        mpp = ctx.enter_context(tc.tile_pool(name="mpp", bufs=3))
        bnd = ctx.enter_context(tc.tile_pool(name="bnd", bufs=2))
        amp = ctx.enter_context(tc.tile_pool(name="amp", bufs=2))
        acp = ctx.enter_context(tc.tile_pool(name="acp", bufs=1))
        wsp = ctx.enter_context(tc.tile_pool(name="wsp", bufs=1))
        otp = ctx.enter_context(tc.tile_pool(name="otp", bufs=1))
        psb = ctx.enter_context(tc.tile_pool(name="psb", bufs=1, space="PSUM"))
        pst = ctx.enter_context(tc.tile_pool(name="pst", bufs=2, space="PSUM"))
        psp = ctx.enter_context(tc.tile_pool(name="psp", bufs=2, space="PSUM"))

        ones = cst.tile([128, 1], bf, tag="ones")
        nc.scalar.dma_start(ones[:], ones_p[:, :])
        identb = cst.tile([128, 128], bf, tag="identb")
        make_identity(nc, identb[:])
        # staircase selector: estep[:, 47-di : 47-di+dn] is a [FCH, dn]
        # matrix whose only nonzero column is column di (all ones) -> matmul
        # with it as lhsT reduces partitions into PSUM row di
        estep = cst.tile([FCH, 95], bf, tag="estep")
        nc.scalar.dma_start(estep[:], estep_p[:, :])

        # ---- setup (once): resident weights/biases, a_scr zero init
        w1t = [[wsp.tile([FCH, FH], bf, tag=f"w1t{k_}{c_}", name=f"w1t{k_}{c_}")
                for c_ in range(NFCH)] for k_ in range(2)]
        for k_ in range(2):
            for c_ in range(NFCH):
                nc.scalar.dma_start(w1t[k_][c_][:],
                                    w1_p[k_, c_ * FCH:(c_ + 1) * FCH, :])
        w2t = [[wsp.tile([128, F], bf, tag=f"w2t{k_}{j_}", name=f"w2t{k_}{j_}")
                for j_ in range(len(FH_BLOCKS))] for k_ in range(2)]
        for k_ in range(2):
            for j_, (ko, kp) in enumerate(FH_BLOCKS):
                nc.scalar.dma_start(w2t[k_][j_][:kp, :],
                                    w2_p[k_, ko:ko + kp, :])
        b1t = [wsp.tile([128, 1], fp, tag=f"b1t{j_}", name=f"b1t{j_}")
               for j_ in range(len(FH_BLOCKS))]
        for j_, (mo, mp_) in enumerate(FH_BLOCKS):
            nc.scalar.dma_start(b1t[j_][:mp_, :], b1_p[mo:mo + mp_])
        b2t = [wsp.tile([FCH, 1], fp, tag=f"b2t{m_}", name=f"b2t{m_}")
               for m_ in range(NFCH)]
        for m_ in range(NFCH):
            nc.scalar.dma_start(b2t[m_][:], b2_p[m_ * FCH:(m_ + 1) * FCH])
        for b in range(SPB):
            ad = a_scr[b]
            for t in range(5):
                lo = t * 128 * WROW
                n_el = min(128 * WROW, WROW * WROW - lo)
                if n_el <= 0:
                    break
                nc.sync.dma_start(AP(ad, lo, [[1, n_el]]),
                                  AP(zeros_p, 0, [[1, n_el]]))

        rep_cm = tc.For_i(0, reps, 1) if reps > 1 else None
        _lo = [False]

        def _open():
            if rep_cm is not None and not _lo[0]:
                rep_cm.__enter__()
                _lo[0] = True

        def _close():
            if _lo[0]:
                rep_cm.__exit__(None, None, None)
                _lo[0] = False

        if phase in ("all", "phase1"):
            _open()

        # ---------------- per-sample state ----------------
        xt_all = [None] * SPB
        xn_all = [None] * SPB
        srow_all = [None] * SPB
        at_all = [None] * SPB
        dinvB_all = [None] * SPB
        psM_all = [None] * SPB  # [DCH, N] bank triple-duty: psS row, band M, deg

        def gen_phase1(b):
            # single 3D DMA fills all five feature chunks side by side
            xtb = xtp.tile([FCH, NFCH * WROW], bf, tag=f"xt{b}", name=f"xt{b}")
            a = xtb[:]
            s = xpt_p[b, :, :]
            nc.scalar.dma_start(
                AP(a.tensor, a.offset,
                   [list(a.ap[0]), [WROW, NFCH], [1, WROW]]),
                AP(s.tensor, s.offset,
                   [[WROW, FCH], [FCH * WROW, NFCH], [1, WROW]]))
            xt = [xtb[:, c * WROW:(c + 1) * WROW] for c in range(NFCH)]
            xt_all[b] = xt
            # xn for cheb: one 3D DMA, issued early on SP queue
            xnb = xpp.tile([128, 4 * F], bf, tag=f"xn{b}", name=f"xn{b}")
            a = xnb[:]
            s = xp_p[b, :, :]
            nc.sync.dma_start(
                AP(a.tensor, a.offset, [list(a.ap[0]), [F, 4], [1, F]]),
                AP(s.tensor, s.offset, [[F, 128], [128 * F, 4], [1, F]]))
            xn = [xnb[:, t * F:(t + 1) * F] for t in range(4)]
            xn_all[b] = xn
            yield
            psMS = psb.tile([DCH, N], fp, tag=f"psM{b}", name=f"psM{b}")
            psM_all[b] = psMS
            for c in range(NFCH):
                nc.tensor.matmul(psMS[0:1, :], ones[:FCH, :], xt[c][:, :N],
                                 start=(c == 0), stop=(c == NFCH - 1))
            srow = bnd.tile([1, WROW], fp, tag=f"srow{b}", name=f"srow{b}")
            nc.gpsimd.memset(srow[:, N:], 0.0)
            nc.scalar.copy(srow[:, :N], psMS[0:1, :])
            srow_all[b] = srow
            yield

        def gen_band(b):
            ad = a_scr[b]
            xt, srow = xt_all[b], srow_all[b]
            psM = psM_all[b]
            d0 = 1
            while d0 <= w:
                dn = min(DCH, w - d0 + 1)
                # offsets [d0, d0+dn): tail POOL_N of them on Pool engine.
                # steps = (engine, offset-batch, chunk); pool steps merged
                # evenly among dve steps so the in-order PSUM consumption
                # chain lets both engines produce concurrently
                n_pool = min(POOL_N, max(0, dn - 1)) if POOL_N > 0 else 0
                n_dve = dn - n_pool
                dsteps, psteps = [], []
                o = 0
                while o < n_dve:
                    nb = min(DVE_BD, n_dve - o)
                    for c in range(NFCH):
                        dsteps.append(("dve", o, nb, c))
                    o += nb
                while o < dn:
                    nb = min(POOL_BD, dn - o)
                    for c in range(NFCH):
                        psteps.append(("pool", o, nb, c))
                    o += nb
                steps = []
                nd, np_ = len(dsteps), len(psteps)
                di_, pi_ = 0, 0
                while di_ < nd or pi_ < np_:
                    # keep pool's consumed fraction slightly ahead
                    if pi_ < np_ and (di_ >= nd or
                                      pi_ * nd <= di_ * np_):
                        steps.append(psteps[pi_])
                        pi_ += 1
                    else:
                        steps.append(dsteps[di_])
                        di_ += 1
                total_mm = dn * NFCH
                mm_done = 0
                for eng, db0, nb, c in steps:
                    tp = mxp if eng == "dve" else mpp
                    bd_ = DVE_BD if eng == "dve" else POOL_BD
                    mxb = tp.tile([FCH, bd_ * N], bf, tag=f"mx_{eng}",
                                  name=f"mx_{eng}")
                    base = xt[c][:, 0:N]
                    in0 = AP(base.tensor, base.offset,
                             [list(base.ap[0]), [0, nb], [1, N]])
                    in1 = AP(base.tensor, base.offset + d0 + db0,
                             [list(base.ap[0]), [1, nb], [1, N]])
                    e = nc.vector if eng == "dve" else nc.gpsimd
                    e.tensor_tensor(out=mxb[:, :nb * N], in0=in0, in1=in1,
                                    op=OP.max)
                    for j in range(nb):
                        di = db0 + j
                        mm_done += 1
                        nc.tensor.matmul(
                            psM[:dn, :],
                            estep[:, 47 - di:47 - di + dn],
                            mxb[:, j * N:(j + 1) * N],
                            start=(mm_done == 1),
                            stop=(mm_done == total_mm))
                    yield
                # epilogue: D = 2M - S_i - S_{i+d}; masks -> abnd
                sv = srow[:, :]
                sshift = bnd.tile([dn, N], fp, tag=f"sshift{b}", bufs=1,
                                  name="sshift")
                nc.sync.dma_start(
                    sshift[:],
                    AP(sv.tensor, sv.offset + d0,
                       [list(sv.ap[0]), [1, dn], [1, N]]))
                sb_t = bnd.tile([dn, N], fp, tag=f"sb{b}", bufs=1, name="sb_t")
                nc.sync.dma_start(
                    sb_t[:],
                    AP(sv.tensor, sv.offset,
                       [list(sv.ap[0]), [0, dn], [1, N]]))
                ashift = bnd.tile([dn, N], fp, tag=f"ashift{b}", bufs=1,
                                  name="ashift")
                nc.scalar.dma_start(ashift[:],
                                    AP(attp_p, b * WROW + d0, [[1, dn], [1, N]]))
                ab_t = bnd.tile([dn, N], fp, tag=f"ab{b}", bufs=1, name="ab_t")
                nc.scalar.dma_start(ab_t[:],
                                    AP(attp_p, b * WROW, [[0, dn], [1, N]]))
                yield
                nc.vector.scalar_tensor_tensor(
                    out=sb_t[:], in0=sb_t[:], scalar=DIST_THRESH, in1=sshift[:],
                    op0=OP.add, op1=OP.add)
                nc.vector.scalar_tensor_tensor(
                    out=sshift[:], in0=psM[:dn, :], scalar=2.0, in1=sb_t[:],
                    op0=OP.mult, op1=OP.is_le)
                nc.vector.tensor_sub(ashift[:], ashift[:], ab_t[:])
                nc.vector.tensor_scalar(ab_t[:], ashift[:], ATT_THRESH, None,
                                        op0=OP.is_le)
                abnd = bnd.tile([dn, N], bf, tag=f"abnd{b}", name="abnd")
                nc.vector.tensor_mul(abnd[:], sshift[:], ab_t[:])
                yield
                # transpose to [128, dn] blocks; skewed-contiguous scatter of
                # the upper triangle: A[i, i+d] for d in [d0, d0+dn) lands at
                # a_scr[i*(WROW+1) + d0 + f], contiguous runs of dn elems
                abT = bnd.tile([128, 4 * DCH], bf, tag=f"abT{b}",
                               name="abT", bufs=2)
                for t in range(4):
                    psTa = pst.tile([128, 128], bf, tag="pst",
                                    name="psTa")
                    nc.tensor.transpose(psTa[:, :dn],
                                        abnd[:, t * 128:(t + 1) * 128],
                                        identb[:dn, :dn])
                    nc.scalar.copy(abT[:, t * DCH:t * DCH + dn],
                                   psTa[:, :dn])
                    yield
                av = abT[:]
                nc.sync.dma_start(
                    AP(ad, d0, [[WROW + 1, 128],
                                [128 * (WROW + 1), 4], [1, dn]]),
                    AP(av.tensor, av.offset,
                       [list(av.ap[0]), [DCH, 4], [1, dn]]))
                yield
                d0 += dn

            # dense readback (upper + zeros elsewhere), rebuild lower
            # triangle from symmetric upper via PE transposes
            atb = amp.tile([128, 4 * N], bf, tag=f"at{b}", name=f"at{b}")
            a = atb[:]
            nc.sync.dma_start(
                AP(a.tensor, a.offset, [list(a.ap[0]), [N, 4], [1, N]]),
                AP(ad, 0, [[WROW, 128], [128 * WROW, 4], [1, N]]))
            at = [atb[:, t * N:(t + 1) * N] for t in range(4)]
            yield
            for t in range(4):
                # diagonal block: lower part = upper(t,t)^T; also add I
                psT2 = pst.tile([128, 128], bf, tag="pst", name="psT2")
                nc.tensor.transpose(psT2[:],
                                    at[t][:, t * 128:(t + 1) * 128],
                                    identb[:])
                nc.vector.tensor_tensor(
                    out=at[t][:, t * 128:(t + 1) * 128],
                    in0=at[t][:, t * 128:(t + 1) * 128],
                    in1=psT2[:], op=OP.add)
                nc.vector.tensor_tensor(
                    out=at[t][:, t * 128:(t + 1) * 128],
                    in0=at[t][:, t * 128:(t + 1) * 128],
                    in1=identb[:], op=OP.add)
                if t > 0:
                    # wedge: rows of block t, cols of block t-1
                    psT3 = pst.tile([128, 128], bf, tag="pst",
                                    name="psT3")
                    nc.tensor.transpose(psT3[:],
                                        at[t - 1][:, t * 128:(t + 1) * 128],
                                        identb[:])
                    nc.vector.tensor_tensor(
                        out=at[t][:, (t - 1) * 128:t * 128],
                        in0=at[t][:, (t - 1) * 128:t * 128],
                        in1=psT3[:], op=OP.add)
                yield
            # deg[j] = colsum (A+I); scale cols by 1/deg
            psM2 = psM_all[b]
            for t in range(4):
                nc.tensor.matmul(psM2[0:1, :], ones[:, :], at[t][:],
                                 start=(t == 0), stop=(t == 3))
            dinvR = bnd.tile([1, N], bf, tag=f"dinvR{b}", name=f"dinvR{b}")
            with nc.allow_low_precision(reason="1/deg fits bf16; deg<=512"):
                nc.vector.reciprocal(dinvR[:], psM2[0:1, :])
            dinvB = amp.tile([128, N], bf, tag=f"dinvB{b}", name=f"dinvB{b}")
            nc.gpsimd.partition_broadcast(dinvB[:], dinvR[:, :])
            dinvB_all[b] = dinvB
            at_all[b] = at
            yield

        def gen_cheb(b):
            xt, at, xn = xt_all[b], at_all[b], xn_all[b]
            dinvB = dinvB_all[b]
            zt = [acp.tile([FCH, N], bf, tag=f"zt{b}{m}", name=f"zt{b}{m}")
                  for m in range(NFCH)]
            for m in range(NFCH):
                psZ = psp.tile([FCH, N], fp, tag=f"mm{b}", name="psZ")
                for t in range(4):
                    nc.tensor.matmul(psZ[:], xn[t][:, m * FCH:(m + 1) * FCH],
                                     at[t][:], start=(t == 0), stop=(t == 3))
                nc.vector.tensor_mul(zt[m][:], psZ[:], dinvB[:FCH, :])
                yield

            ht = [acp.tile([128, N], bf, tag=f"ht{b}{k}", name=f"ht{b}{k}")
                  for k in range(len(FH_BLOCKS))]
            for k, (mo, mp_) in enumerate(FH_BLOCKS):
                psH = psp.tile([128, N], fp, tag=f"mm{b}", name="psH")
                for c in range(NFCH):
                    nc.tensor.matmul(psH[:mp_, :], w1t[0][c][:, mo:mo + mp_],
                                     xt[c][:, :N], start=(c == 0), stop=False)
                for c in range(NFCH):
                    nc.tensor.matmul(psH[:mp_, :], w1t[1][c][:, mo:mo + mp_],
                                     zt[c][:], start=False,
                                     stop=(c == NFCH - 1))
                nc.scalar.activation(ht[k][:mp_, :], psH[:mp_, :], AF.Relu,
                                     bias=b1t[k][:mp_, :], scale=1.0)
                yield

            qt = [acp.tile([128, N], bf, tag=f"qt{b}{k}", name=f"qt{b}{k}")
                  for k in range(len(FH_BLOCKS))]
            for k, (mo, mp_) in enumerate(FH_BLOCKS):
                psQ = psp.tile([128, N], fp, tag=f"mm{b}", name="psQ")
                for t in range(4):
                    psT = pst.tile([128, 128], bf, tag="pst", name="psT")
                    nc.tensor.transpose(
                        psT[:, :mp_],
                        ht[k][:mp_, t * 128:(t + 1) * 128],
                        identb[:mp_, :mp_])
                    hb = bnd.tile([128, 128], bf, tag=f"hb{b}", bufs=3,
                                  name="hb")
                    nc.scalar.copy(hb[:, :mp_], psT[:, :mp_])
                    nc.tensor.matmul(psQ[:mp_, :], hb[:, :mp_], at[t][:],
                                     start=(t == 0), stop=(t == 3))
                nc.vector.tensor_mul(qt[k][:mp_, :], psQ[:mp_, :],
                                     dinvB[:mp_, :])
                yield

            for m in range(NFCH):
                psO = psp.tile([FCH, N], fp, tag=f"mm{b}", name="psO")
                for k, (ko, kp) in enumerate(FH_BLOCKS):
                    nc.tensor.matmul(psO[:],
                                     w2t[0][k][:kp, m * FCH:(m + 1) * FCH],
                                     ht[k][:kp, :], start=(k == 0), stop=False)
                for k, (ko, kp) in enumerate(FH_BLOCKS):
                    nc.tensor.matmul(psO[:],
                                     w2t[1][k][:kp, m * FCH:(m + 1) * FCH],
                                     qt[k][:kp, :], start=False,
                                     stop=(k == len(FH_BLOCKS) - 1))
                ot = otp.tile([FCH, N], fp, tag=f"ot{b}", name="ot")
                nc.scalar.activation(ot[:], psO[:], AF.Relu, bias=b2t[m][:],
                                     scale=1.0)
                nc.sync.dma_start(out_p[b, m * FCH:(m + 1) * FCH, :], ot[:])
                yield

        def rr(*gens):
            gens = list(gens)
            while gens:
                g = gens.pop(0)
                if next(g, StopIteration) is not StopIteration:
                    gens.append(g)

        def stagger(g_a, g_b, ratio=2):
            # drive g_a `ratio` steps per g_b step until both exhausted
            done_a = done_b = False
            while not (done_a and done_b):
                for _ in range(ratio):
                    if not done_a:
                        done_a = next(g_a, StopIteration) is StopIteration
                if not done_b:
                    done_b = next(g_b, StopIteration) is StopIteration

        if phase == "all":
            rr(gen_phase1(0), gen_phase1(1))
            for _ in gen_band(0):
                pass
            stagger(gen_band(1), gen_cheb(0), ratio=2)
            for _ in gen_cheb(1):
                pass
        else:
            rr(gen_phase1(0), gen_phase1(1))
            if phase == "phase1":
                _close()
            if phase == "band":
                _open()
            rr(gen_band(0), gen_band(1))
            if phase == "band":
                _close()
            if phase == "cheb":
                _open()
            rr(gen_cheb(0), gen_cheb(1))
            if phase == "cheb":
                _close()
            if phase == "empty":
                _open()
                dummy = bnd.tile([1, 1], fp, tag="dummy", name="dummy")
                nc.scalar.copy(dummy[:], ones[:1, :].bitcast(fp))
                _close()
        _close()

    if not nc.is_finalized():
        nc.finalize()
    return nc, WROW


def _prepare(x4, attention):
    """Host prep: flatten, sort by attention, compute band width, pad."""
    X = np.ascontiguousarray(x4.reshape(B, N, F), dtype=np.float32)
    att = np.ascontiguousarray(attention[:, :, 0, 0], dtype=np.float32)
    perms = np.argsort(att, axis=1, kind="stable")
    attp = np.take_along_axis(att, perms, axis=1)
    a64 = attp.astype(np.float64)
    w = 1
    for bi in range(B):
        for d in range(1, N):
            if np.min(a64[bi, d:] - a64[bi, :-d]) <= ATT_THRESH + 1e-6:
                w = max(w, d)
            else:
                break  # windows only widen with d
    w = min(w, N - 1)
    Xp = np.take_along_axis(X, perms[:, :, None], axis=1)
    return Xp, attp, perms, w


def _make_runner(nc):
    """Compile the Bass program into a reusable 8-core sharded jax callable.

    Mirrors concourse.bass2jax.run_bass_via_pjrt's multi-core branch, but
    returns the compiled callable so repeated executions can be timed.
    """
    import jax
    from jax.sharding import Mesh, PartitionSpec
    from jax.experimental.shard_map import shard_map
    from concourse import bass2jax, mybir

    bass2jax.install_neuronx_cc_hook()

    in_names, out_names, out_avals, zero_outs = [], [], [], []
    partition_name = (nc.partition_id_tensor.name
                      if nc.partition_id_tensor else None)
    for alloc in nc.m.functions[0].allocations:
        if not isinstance(alloc, mybir.MemoryLocationSet):
            continue
        name = alloc.memorylocations[0].name
        if alloc.kind == "ExternalInput":
            if name != partition_name:
                in_names.append(name)
        elif alloc.kind == "ExternalOutput":
            shape = tuple(alloc.tensor_shape)
            dtype = mybir.dt.np(alloc.dtype)
            out_names.append(name)
            out_avals.append(jax.core.ShapedArray(shape, dtype))
            zero_outs.append(np.zeros(shape, dtype))
    n_params = len(in_names)
    n_outs = len(out_avals)
    in_names = in_names + out_names
    if partition_name is not None:
        in_names.append(partition_name)
    donate = tuple(range(n_params, n_params + n_outs))

    def _body(*args):
        operands = list(args)
        if partition_name is not None:
            operands.append(bass2jax.partition_id_tensor())
        outs = bass2jax._bass_exec_p.bind(
            *operands,
            out_avals=tuple(out_avals),
            in_names=tuple(in_names),
            out_names=tuple(out_names),
            lowering_input_output_aliases=(),
            sim_require_finite=True,
            sim_require_nnan=True,
            nc=nc,
        )
        return tuple(outs)

    devices = jax.devices()[:NCORES]
    mesh = Mesh(np.asarray(devices), ("core",))
    sharded = jax.jit(
        shard_map(_body, mesh=mesh,
                  in_specs=(PartitionSpec("core"),) * (n_params + n_outs),
                  out_specs=(PartitionSpec("core"),) * n_outs,
                  check_rep=False),
        donate_argnums=donate, keep_unused=True)

    param_order = in_names[:n_params]

    def run(in_maps):
        concat_in = [
            np.concatenate([np.asarray(in_maps[c][nm]) for c in range(NCORES)],
                           axis=0)
            for nm in param_order
        ]
        concat_zeros = [np.zeros((NCORES * z.shape[0], *z.shape[1:]), z.dtype)
                        for z in zero_outs]
        out_arrs = jax.block_until_ready(sharded(*concat_in, *concat_zeros))
        return [
            {nm: np.asarray(out_arrs[i]).reshape(NCORES, *out_avals[i].shape)[c]
             for i, nm in enumerate(out_names)}
            for c in range(NCORES)
        ]

    return {"run": run, "sharded": sharded, "param_order": param_order,
            "zero_outs": zero_outs, "out_names": out_names,
            "out_avals": out_avals, "mesh": mesh}


def _get_runner(w, reps=None):
    import os
    mm = os.environ.get("KERNEL_MM_DTYPE", "bf16")
    if reps is None:
        reps = int(os.environ.get("KERNEL_REPS", "1"))
    phase = os.environ.get("KERNEL_PHASE", "all")
    key = (w, mm, reps, phase,
           os.environ.get("KERNEL_DVE_BD", ""),
           os.environ.get("KERNEL_POOL_N", ""),
           os.environ.get("KERNEL_POOL_BD", ""))
    if key not in _prog_cache:
        nc, WROW = _build_program(w, mm=mm, reps=reps, phase=phase)
        _prog_cache[key] = (_make_runner(nc), WROW)
    return _prog_cache[key]


def kernel(x4, attention, W1, b1, W2, b2):
    import ml_dtypes
    bf16 = ml_dtypes.bfloat16
    Xp, attp, perms, w = _prepare(x4, attention)
    runner, WROW = _get_runner(w, reps=1)

    Xpb = Xp.astype(bf16)
    xpt = np.zeros((B, F, WROW), bf16)
    xpt[:, :, :N] = Xpb.transpose(0, 2, 1)
    attp_pad = np.full((B, WROW), 1e9, np.float32)
    attp_pad[:, :N] = attp

    W1b = np.ascontiguousarray(W1, dtype=np.float32).astype(bf16)
    W2b = np.ascontiguousarray(W2, dtype=np.float32).astype(bf16)
    b1 = np.ascontiguousarray(b1, dtype=np.float32)
    b2 = np.ascontiguousarray(b2, dtype=np.float32)

    c_ones = np.ones((128, 1), bf16)
    c_estep = np.zeros((FCH, 95), bf16)
    c_estep[:, 47] = 1.0
    c_zeros = np.zeros((128, WROW), bf16)

    in_maps = []
    for c in range(NCORES):
        sl = slice(c * SPB, (c + 1) * SPB)
        in_maps.append({
            "xp": np.ascontiguousarray(Xpb[sl]),
            "xpt": np.ascontiguousarray(xpt[sl]),
            "attp": np.ascontiguousarray(attp_pad[sl]),
            "w1": W1b, "b1": b1, "w2": W2b, "b2": b2,
            "c_ones": c_ones, "c_estep": c_estep, "c_zeros": c_zeros,
        })

    results = runner["run"](in_maps)
    globals()["last_in_maps"] = in_maps
    globals()["last_runner"] = runner

    inv = np.argsort(perms, axis=1)
    out = np.empty((B, N, F), np.float32)
    for c in range(NCORES):
        o = results[c]["outT"]  # [SPB, F, N]
        for s in range(SPB):
            bi = c * SPB + s
            out[bi] = o[s].T[inv[bi]]
    return out


# revision 32
# speedup vs baseline: 1.2556x; 1.1082x over previous
"""Trainium2 Bass kernel: batched ChebConv GNN with L1-distance adjacency.

Pipeline per sample (N=512 nodes, F=625 features):
  1. Sort nodes by attention (host). All pairs with |att_i-att_j| <= 0.05
     then lie within a rank band |i-j| <= w (w computed exactly on host).
  2. Banded pairwise L1 distances on device via the exact identity
     sum_f |a-b| = 2*sum_f max(a,b) - S_i - S_j  (S = row sums). The max
     runs on DVE in bf16 (2x DVE throughput; masks flip on only 0.08% of
     band pairs vs fp32); the feature reduction is a PE staircase matmul
     into one PSUM row per band offset.
  3. Threshold masks -> banded adjacency [w, N] -> PE-transpose to
     [128, w] tiles -> skewed-contiguous DMA into a dense upper-triangle
     DRAM matrix (168B runs, not 4B diagonal elements). Lower triangle
     rebuilt in SBUF from the readback via PE transposes (A symmetric).
  4. deg via PE column-sum matmuls; reciprocal row broadcast; ChebConv x2
     as bf16 PE matmuls in transposed layouts.
Data parallel over batch: 16 samples, 8 cores, 2 samples/core, the two
samples unit-interleaved so DVE/Pool/PE/Act all stay fed.
"""

import numpy as np
from contextlib import ExitStack

B, N = 16, 512
F, FH = 625, 937
FCH, NFCH = 125, 5  # feature chunks: 5 x 125 = 625
NCORES = 8
SPB = B // NCORES  # samples per core
DIST_THRESH, ATT_THRESH = 180.0, 0.05
DCH = 48  # band offsets per PSUM group

# FH row blocks (7x128 + 41)
FH_BLOCKS = [(o, min(128, FH - o)) for o in range(0, FH, 128)]

_prog_cache = {}


def _build_program(w, mm="bf16", reps=1, phase="all"):
    """Build the SPMD Bass program for band half-width w. Returns (nc, WROW).

    phase: which section the reps hardware loop wraps ("all", "phase1",
    "band", "cheb", "empty") — ablation instrument; result stays correct.
    """
    import os as _os
    import concourse.bass as bass
    import concourse.bacc as bacc
    import concourse.mybir as mybir
    import concourse.tile as tile
    from concourse.masks import make_identity

    dt = mybir.dt
    fp = dt.float32
    bf = dt.bfloat16
    AF = mybir.ActivationFunctionType
    OP = mybir.AluOpType
    AP = bass.AP

    DVE_BD = int(_os.environ.get("KERNEL_DVE_BD", "6"))
    POOL_N = int(_os.environ.get("KERNEL_POOL_N", "0"))  # offsets/chunk on Pool
    POOL_BD = int(_os.environ.get("KERNEL_POOL_BD", "6"))

    padw = ((w + 7) // 8) * 8
    WROW = N + padw  # padded row width for xt rows / a_scr cols

    nc = bacc.Bacc()
    xp_p = nc.declare_dram_parameter("xp", [SPB, N, F], bf, isOutput=False)
    xpt_p = nc.declare_dram_parameter("xpt", [SPB, F, WROW], bf, isOutput=False)
    attp_p = nc.declare_dram_parameter("attp", [SPB, WROW], fp, isOutput=False)
    w1_p = nc.declare_dram_parameter("w1", [2, F, FH], bf, isOutput=False)
    b1_p = nc.declare_dram_parameter("b1", [FH], fp, isOutput=False)
    w2_p = nc.declare_dram_parameter("w2", [2, FH, F], bf, isOutput=False)
    b2_p = nc.declare_dram_parameter("b2", [F], fp, isOutput=False)
    out_p = nc.declare_dram_parameter("outT", [SPB, F, N], fp, isOutput=True)
    ones_p = nc.declare_dram_parameter("c_ones", [128, 1], bf, isOutput=False)
    estep_p = nc.declare_dram_parameter("c_estep", [FCH, 95], bf, isOutput=False)
    zeros_p = nc.declare_dram_parameter("c_zeros", [128, WROW], bf, isOutput=False)

    # internal DRAM scratch: dense adjacency rows, one per sample slot
    a_scr = [nc.dram_tensor(f"a_scr{b}", [WROW * WROW], bf) for b in range(SPB)]

    with tile.TileContext(nc) as tc, ExitStack() as ctx:
        cst = ctx.enter_context(tc.tile_pool(name="cst", bufs=1))
        xtp = ctx.enter_context(tc.tile_pool(name="xtp", bufs=2))
        xpp = ctx.enter_context(tc.tile_pool(name="xpp", bufs=1))
        mxp = ctx.enter_context(tc.tile_pool(name="mxp", bufs=4))
        mpp = ctx.enter_context(tc.tile_pool(name="mpp", bufs=3))
        bnd = ctx.enter_context(tc.tile_pool(name="bnd", bufs=2))
        amp = ctx.enter_context(tc.tile_pool(name="amp", bufs=1))
        acp = ctx.enter_context(tc.tile_pool(name="acp", bufs=1))
        wsp = ctx.enter_context(tc.tile_pool(name="wsp", bufs=1))
        otp = ctx.enter_context(tc.tile_pool(name="otp", bufs=1))
        psb = ctx.enter_context(tc.tile_pool(name="psb", bufs=1, space="PSUM"))
        pst = ctx.enter_context(tc.tile_pool(name="pst", bufs=2, space="PSUM"))
        psp = ctx.enter_context(tc.tile_pool(name="psp", bufs=2, space="PSUM"))

        ones = cst.tile([128, 1], bf, tag="ones")
        nc.scalar.dma_start(ones[:], ones_p[:, :])
        identb = cst.tile([128, 128], bf, tag="identb")
        make_identity(nc, identb[:])
        # staircase selector: estep[:, 47-di : 47-di+dn] is a [FCH, dn]
        # matrix whose only nonzero column is column di (all ones) -> matmul
        # with it as lhsT reduces partitions into PSUM row di
        estep = cst.tile([FCH, 95], bf, tag="estep")
        nc.scalar.dma_start(estep[:], estep_p[:, :])

        # ---- setup (once): resident weights/biases, a_scr zero init
        w1t = [[wsp.tile([FCH, FH], bf, tag=f"w1t{k_}{c_}", name=f"w1t{k_}{c_}")
                for c_ in range(NFCH)] for k_ in range(2)]
        for k_ in range(2):
            for c_ in range(NFCH):
                nc.scalar.dma_start(w1t[k_][c_][:],
                                    w1_p[k_, c_ * FCH:(c_ + 1) * FCH, :])
        w2t = [[wsp.tile([128, F], bf, tag=f"w2t{k_}{j_}", name=f"w2t{k_}{j_}")
                for j_ in range(len(FH_BLOCKS))] for k_ in range(2)]
        for k_ in range(2):
            for j_, (ko, kp) in enumerate(FH_BLOCKS):
                nc.scalar.dma_start(w2t[k_][j_][:kp, :],
                                    w2_p[k_, ko:ko + kp, :])
        b1t = [wsp.tile([128, 1], fp, tag=f"b1t{j_}", name=f"b1t{j_}")
               for j_ in range(len(FH_BLOCKS))]
        for j_, (mo, mp_) in enumerate(FH_BLOCKS):
            nc.scalar.dma_start(b1t[j_][:mp_, :], b1_p[mo:mo + mp_])
        b2t = [wsp.tile([FCH, 1], fp, tag=f"b2t{m_}", name=f"b2t{m_}")
               for m_ in range(NFCH)]
        for m_ in range(NFCH):
            nc.scalar.dma_start(b2t[m_][:], b2_p[m_ * FCH:(m_ + 1) * FCH])
        for b in range(SPB):
            ad = a_scr[b]
            for t in range(5):
                lo = t * 128 * WROW
                n_el = min(128 * WROW, WROW * WROW - lo)
                if n_el <= 0:
                    break
                nc.sync.dma_start(AP(ad, lo, [[1, n_el]]),
                                  AP(zeros_p, 0, [[1, n_el]]))

        rep_cm = tc.For_i(0, reps, 1) if reps > 1 else None
        _lo = [False]

        def _open():
            if rep_cm is not None and not _lo[0]:
                rep_cm.__enter__()
                _lo[0] = True

        def _close():
            if _lo[0]:
                rep_cm.__exit__(None, None, None)
                _lo[0] = False

        if phase in ("all", "phase1"):
            _open()

        # ---------------- per-sample state ----------------
        xt_all = [None] * SPB
        xn_all = [None] * SPB
        srow_all = [None] * SPB
        at_all = [None] * SPB
        dinvB_all = [None] * SPB
        psM_all = [None] * SPB  # [DCH, N] bank triple-duty: psS row, band M, deg

        def gen_phase1(b):
            # single 3D DMA fills all five feature chunks side by side
            xtb = xtp.tile([FCH, NFCH * WROW], bf, tag=f"xt{b}", name=f"xt{b}")
            a = xtb[:]
            s = xpt_p[b, :, :]
            nc.scalar.dma_start(
                AP(a.tensor, a.offset,
                   [list(a.ap[0]), [WROW, NFCH], [1, WROW]]),
                AP(s.tensor, s.offset,
                   [[WROW, FCH], [FCH * WROW, NFCH], [1, WROW]]))
            xt = [xtb[:, c * WROW:(c + 1) * WROW] for c in range(NFCH)]
            xt_all[b] = xt
            # xn for cheb: one 3D DMA, issued early on SP queue
            xnb = xpp.tile([128, 4 * F], bf, tag=f"xn{b}", name=f"xn{b}")
            a = xnb[:]
            s = xp_p[b, :, :]
            nc.sync.dma_start(
                AP(a.tensor, a.offset, [list(a.ap[0]), [F, 4], [1, F]]),
                AP(s.tensor, s.offset, [[F, 128], [128 * F, 4], [1, F]]))
            xn = [xnb[:, t * F:(t + 1) * F] for t in range(4)]
            xn_all[b] = xn
            yield
            psMS = psb.tile([DCH, N], fp, tag=f"psM{b}", name=f"psM{b}")
            psM_all[b] = psMS
            for c in range(NFCH):
                nc.tensor.matmul(psMS[0:1, :], ones[:FCH, :], xt[c][:, :N],
                                 start=(c == 0), stop=(c == NFCH - 1))
            srow = bnd.tile([1, WROW], fp, tag=f"srow{b}", name=f"srow{b}")
            nc.gpsimd.memset(srow[:, N:], 0.0)
            nc.scalar.copy(srow[:, :N], psMS[0:1, :])
            srow_all[b] = srow
            yield

        def gen_band(b):
            ad = a_scr[b]
            xt, srow = xt_all[b], srow_all[b]
            psM = psM_all[b]
            d0 = 1
            while d0 <= w:
                dn = min(DCH, w - d0 + 1)
                # offsets [d0, d0+dn): tail POOL_N of them on Pool engine.
                # steps = (engine, offset-batch, chunk); pool steps merged
                # evenly among dve steps so the in-order PSUM consumption
                # chain lets both engines produce concurrently
                n_pool = min(POOL_N, max(0, dn - 1)) if POOL_N > 0 else 0
                n_dve = dn - n_pool
                dsteps, psteps = [], []
                o = 0
                while o < n_dve:
                    nb = min(DVE_BD, n_dve - o)
                    for c in range(NFCH):
                        dsteps.append(("dve", o, nb, c))
                    o += nb
                while o < dn:
                    nb = min(POOL_BD, dn - o)
                    for c in range(NFCH):
                        psteps.append(("pool", o, nb, c))
                    o += nb
                steps = []
                nd, np_ = len(dsteps), len(psteps)
                di_, pi_ = 0, 0
                while di_ < nd or pi_ < np_:
                    # keep pool's consumed fraction slightly ahead
                    if pi_ < np_ and (di_ >= nd or
                                      pi_ * nd <= di_ * np_):
                        steps.append(psteps[pi_])
                        pi_ += 1
                    else:
                        steps.append(dsteps[di_])
                        di_ += 1
                total_mm = dn * NFCH
                mm_done = 0
                for eng, db0, nb, c in steps:
                    tp = mxp if eng == "dve" else mpp
                    bd_ = DVE_BD if eng == "dve" else POOL_BD
                    mxb = tp.tile([FCH, bd_ * N], bf, tag=f"mx_{eng}",
                                  name=f"mx_{eng}")
                    base = xt[c][:, 0:N]
                    in0 = AP(base.tensor, base.offset,
                             [list(base.ap[0]), [0, nb], [1, N]])
                    in1 = AP(base.tensor, base.offset + d0 + db0,
                             [list(base.ap[0]), [1, nb], [1, N]])
                    e = nc.vector if eng == "dve" else nc.gpsimd
                    e.tensor_tensor(out=mxb[:, :nb * N], in0=in0, in1=in1,
                                    op=OP.max)
                    for j in range(nb):
                        di = db0 + j
                        mm_done += 1
                        nc.tensor.matmul(
                            psM[:dn, :],
                            estep[:, 47 - di:47 - di + dn],
                            mxb[:, j * N:(j + 1) * N],
                            start=(mm_done == 1),
                            stop=(mm_done == total_mm))
                    yield
                # epilogue: D = 2M - S_i - S_{i+d}; masks -> abnd
                sv = srow[:, :]
                sshift = bnd.tile([dn, N], fp, tag=f"sshift{b}", bufs=1,
                                  name="sshift")
                nc.sync.dma_start(
                    sshift[:],
                    AP(sv.tensor, sv.offset + d0,
                       [list(sv.ap[0]), [1, dn], [1, N]]))
                sb_t = bnd.tile([dn, N], fp, tag=f"sb{b}", bufs=1, name="sb_t")
                nc.sync.dma_start(
                    sb_t[:],
                    AP(sv.tensor, sv.offset,
                       [list(sv.ap[0]), [0, dn], [1, N]]))
                ashift = bnd.tile([dn, N], fp, tag=f"ashift{b}", bufs=1,
                                  name="ashift")
                nc.scalar.dma_start(ashift[:],
                                    AP(attp_p, b * WROW + d0, [[1, dn], [1, N]]))
                ab_t = bnd.tile([dn, N], fp, tag=f"ab{b}", bufs=1, name="ab_t")
                nc.scalar.dma_start(ab_t[:],
                                    AP(attp_p, b * WROW, [[0, dn], [1, N]]))
                yield
                nc.vector.scalar_tensor_tensor(
                    out=sb_t[:], in0=sb_t[:], scalar=DIST_THRESH, in1=sshift[:],
                    op0=OP.add, op1=OP.add)
                nc.vector.scalar_tensor_tensor(
                    out=sshift[:], in0=psM[:dn, :], scalar=2.0, in1=sb_t[:],
                    op0=OP.mult, op1=OP.is_le)
                nc.vector.tensor_sub(ashift[:], ashift[:], ab_t[:])
                nc.vector.tensor_scalar(ab_t[:], ashift[:], ATT_THRESH, None,
                                        op0=OP.is_le)
                abnd = bnd.tile([dn, N], bf, tag=f"abnd{b}", name="abnd")
                nc.vector.tensor_mul(abnd[:], sshift[:], ab_t[:])
                yield
                # transpose to [128, dn] blocks; skewed-contiguous scatter of
                # the upper triangle: A[i, i+d] for d in [d0, d0+dn) lands at
                # a_scr[i*(WROW+1) + d0 + f], contiguous runs of dn elems
                abT = bnd.tile([128, 4 * DCH], bf, tag=f"abT{b}",
                               name="abT", bufs=2)
                for t in range(4):
                    psTa = pst.tile([128, 128], bf, tag="pst",
                                    name="psTa")
                    nc.tensor.transpose(psTa[:, :dn],
                                        abnd[:, t * 128:(t + 1) * 128],
                                        identb[:dn, :dn])
                    nc.scalar.copy(abT[:, t * DCH:t * DCH + dn],
                                   psTa[:, :dn])
                    yield
                av = abT[:]
                nc.sync.dma_start(
                    AP(ad, d0, [[WROW + 1, 128],
                                [128 * (WROW + 1), 4], [1, dn]]),
                    AP(av.tensor, av.offset,
                       [list(av.ap[0]), [DCH, 4], [1, dn]]))
                yield
                d0 += dn

            # dense readback (upper + zeros elsewhere), rebuild lower
            # triangle from symmetric upper via PE transposes
            atb = amp.tile([128, 4 * N], bf, tag=f"at{b}", name=f"at{b}")
            a = atb[:]
            nc.sync.dma_start(
                AP(a.tensor, a.offset, [list(a.ap[0]), [N, 4], [1, N]]),
                AP(ad, 0, [[WROW, 128], [128 * WROW, 4], [1, N]]))
            at = [atb[:, t * N:(t + 1) * N] for t in range(4)]
            yield
            for t in range(4):
                # diagonal block: lower part = upper(t,t)^T; also add I
                psT2 = pst.tile([128, 128], bf, tag="pst", name="psT2")
                nc.tensor.transpose(psT2[:],
                                    at[t][:, t * 128:(t + 1) * 128],
                                    identb[:])
                nc.vector.tensor_tensor(
                    out=at[t][:, t * 128:(t + 1) * 128],
                    in0=at[t][:, t * 128:(t + 1) * 128],
                    in1=psT2[:], op=OP.add)
                nc.vector.tensor_tensor(
                    out=at[t][:, t * 128:(t + 1) * 128],
                    in0=at[t][:, t * 128:(t + 1) * 128],
                    in1=identb[:], op=OP.add)
                if t > 0:
                    # wedge: rows of block t, cols of block t-1
                    psT3 = pst.tile([128, 128], bf, tag="pst",
                                    name="psT3")
                    nc.tensor.transpose(psT3[:],
                                        at[t - 1][:, t * 128:(t + 1) * 128],
                                        identb[:])
                    nc.vector.tensor_tensor(
                        out=at[t][:, (t - 1) * 128:t * 128],
                        in0=at[t][:, (t - 1) * 128:t * 128],
                        in1=psT3[:], op=OP.add)
                yield
            # deg[j] = colsum (A+I); scale cols by 1/deg
            psM2 = psM_all[b]
            for t in range(4):
                nc.tensor.matmul(psM2[0:1, :], ones[:, :], at[t][:],
                                 start=(t == 0), stop=(t == 3))
            dinvR = bnd.tile([1, N], fp, tag=f"dinvR{b}", name=f"dinvR{b}")
            nc.vector.reciprocal(dinvR[:], psM2[0:1, :])
            dinvB = amp.tile([128, N], fp, tag=f"dinvB{b}", name=f"dinvB{b}")
            nc.gpsimd.partition_broadcast(dinvB[:], dinvR[:, :])
            dinvB_all[b] = dinvB
            at_all[b] = at
            yield

        def gen_cheb(b):
            xt, at, xn = xt_all[b], at_all[b], xn_all[b]
            dinvB = dinvB_all[b]
            zt = [acp.tile([FCH, N], bf, tag=f"zt{b}{m}", name=f"zt{b}{m}")
                  for m in range(NFCH)]
            for m in range(NFCH):
                psZ = psp.tile([FCH, N], fp, tag=f"mm{b}", name="psZ")
                for t in range(4):
                    nc.tensor.matmul(psZ[:], xn[t][:, m * FCH:(m + 1) * FCH],
                                     at[t][:], start=(t == 0), stop=(t == 3))
                nc.vector.tensor_mul(zt[m][:], psZ[:], dinvB[:FCH, :])
                yield

            ht = [acp.tile([128, N], bf, tag=f"ht{b}{k}", name=f"ht{b}{k}")
                  for k in range(len(FH_BLOCKS))]
            for k, (mo, mp_) in enumerate(FH_BLOCKS):
                psH = psp.tile([128, N], fp, tag=f"mm{b}", name="psH")
                for c in range(NFCH):
                    nc.tensor.matmul(psH[:mp_, :], w1t[0][c][:, mo:mo + mp_],
                                     xt[c][:, :N], start=(c == 0), stop=False)
                for c in range(NFCH):
                    nc.tensor.matmul(psH[:mp_, :], w1t[1][c][:, mo:mo + mp_],
                                     zt[c][:], start=False,
                                     stop=(c == NFCH - 1))
                nc.scalar.activation(ht[k][:mp_, :], psH[:mp_, :], AF.Relu,
                                     bias=b1t[k][:mp_, :], scale=1.0)
                yield

            qt = [acp.tile([128, N], bf, tag=f"qt{b}{k}", name=f"qt{b}{k}")
                  for k in range(len(FH_BLOCKS))]
            for k, (mo, mp_) in enumerate(FH_BLOCKS):
                psQ = psp.tile([128, N], fp, tag=f"mm{b}", name="psQ")
                for t in range(4):
                    psT = pst.tile([128, 128], bf, tag="pst", name="psT")
                    nc.tensor.transpose(
                        psT[:, :mp_],
                        ht[k][:mp_, t * 128:(t + 1) * 128],
                        identb[:mp_, :mp_])
                    hb = bnd.tile([128, 128], bf, tag=f"hb{b}", bufs=3,
                                  name="hb")
                    nc.scalar.copy(hb[:, :mp_], psT[:, :mp_])
                    nc.tensor.matmul(psQ[:mp_, :], hb[:, :mp_], at[t][:],
                                     start=(t == 0), stop=(t == 3))
                nc.vector.tensor_mul(qt[k][:mp_, :], psQ[:mp_, :],
                                     dinvB[:mp_, :])
                yield

            for m in range(NFCH):
                psO = psp.tile([FCH, N], fp, tag=f"mm{b}", name="psO")
                for k, (ko, kp) in enumerate(FH_BLOCKS):
                    nc.tensor.matmul(psO[:],
                                     w2t[0][k][:kp, m * FCH:(m + 1) * FCH],
                                     ht[k][:kp, :], start=(k == 0), stop=False)
                for k, (ko, kp) in enumerate(FH_BLOCKS):
                    nc.tensor.matmul(psO[:],
                                     w2t[1][k][:kp, m * FCH:(m + 1) * FCH],
                                     qt[k][:kp, :], start=False,
                                     stop=(k == len(FH_BLOCKS) - 1))
                ot = otp.tile([FCH, N], fp, tag=f"ot{b}", name="ot")
                nc.scalar.activation(ot[:], psO[:], AF.Relu, bias=b2t[m][:],
                                     scale=1.0)
                nc.sync.dma_start(out_p[b, m * FCH:(m + 1) * FCH, :], ot[:])
                yield

        def rr(*gens):
            gens = list(gens)
            while gens:
                g = gens.pop(0)
                if next(g, StopIteration) is not StopIteration:
                    gens.append(g)

        def stagger(g_a, g_b, ratio=2):
            # drive g_a `ratio` steps per g_b step until both exhausted
            done_a = done_b = False
            while not (done_a and done_b):
                for _ in range(ratio):
                    if not done_a:
                        done_a = next(g_a, StopIteration) is StopIteration
                if not done_b:
                    done_b = next(g_b, StopIteration) is StopIteration

        if phase == "all":
            rr(gen_phase1(0), gen_phase1(1))
            for _ in gen_band(0):
                pass
            stagger(gen_band(1), gen_cheb(0), ratio=2)
            for _ in gen_cheb(1):
                pass
        else:
            rr(gen_phase1(0), gen_phase1(1))
            if phase == "phase1":
                _close()
            if phase == "band":
                _open()
            rr(gen_band(0), gen_band(1))
            if phase == "band":
                _close()
            if phase == "cheb":
                _open()
            rr(gen_cheb(0), gen_cheb(1))
            if phase == "cheb":
                _close()
            if phase == "empty":
                _open()
                dummy = bnd.tile([1, 1], fp, tag="dummy", name="dummy")
                nc.scalar.copy(dummy[:], ones[:1, :].bitcast(fp))
                _close()
        _close()

    if not nc.is_finalized():
        nc.finalize()
    return nc, WROW


def _prepare(x4, attention):
    """Host prep: flatten, sort by attention, compute band width, pad."""
    X = np.ascontiguousarray(x4.reshape(B, N, F), dtype=np.float32)
    att = np.ascontiguousarray(attention[:, :, 0, 0], dtype=np.float32)
    perms = np.argsort(att, axis=1, kind="stable")
    attp = np.take_along_axis(att, perms, axis=1)
    a64 = attp.astype(np.float64)
    w = 1
    for bi in range(B):
        for d in range(1, N):
            if np.min(a64[bi, d:] - a64[bi, :-d]) <= ATT_THRESH + 1e-6:
                w = max(w, d)
            else:
                break  # windows only widen with d
    w = min(w, N - 1)
    Xp = np.take_along_axis(X, perms[:, :, None], axis=1)
    return Xp, attp, perms, w


def _make_runner(nc):
    """Compile the Bass program into a reusable 8-core sharded jax callable.

    Mirrors concourse.bass2jax.run_bass_via_pjrt's multi-core branch, but
    returns the compiled callable so repeated executions can be timed.
    """
    import jax
    from jax.sharding import Mesh, PartitionSpec
    from jax.experimental.shard_map import shard_map
    from concourse import bass2jax, mybir

    bass2jax.install_neuronx_cc_hook()

    in_names, out_names, out_avals, zero_outs = [], [], [], []
    partition_name = (nc.partition_id_tensor.name
                      if nc.partition_id_tensor else None)
    for alloc in nc.m.functions[0].allocations:
        if not isinstance(alloc, mybir.MemoryLocationSet):
            continue
        name = alloc.memorylocations[0].name
        if alloc.kind == "ExternalInput":
            if name != partition_name:
                in_names.append(name)
        elif alloc.kind == "ExternalOutput":
            shape = tuple(alloc.tensor_shape)
            dtype = mybir.dt.np(alloc.dtype)
            out_names.append(name)
            out_avals.append(jax.core.ShapedArray(shape, dtype))
            zero_outs.append(np.zeros(shape, dtype))
    n_params = len(in_names)
    n_outs = len(out_avals)
    in_names = in_names + out_names
    if partition_name is not None:
        in_names.append(partition_name)
    donate = tuple(range(n_params, n_params + n_outs))

    def _body(*args):
        operands = list(args)
        if partition_name is not None:
            operands.append(bass2jax.partition_id_tensor())
        outs = bass2jax._bass_exec_p.bind(
            *operands,
            out_avals=tuple(out_avals),
            in_names=tuple(in_names),
            out_names=tuple(out_names),
            lowering_input_output_aliases=(),
            sim_require_finite=True,
            sim_require_nnan=True,
            nc=nc,
        )
        return tuple(outs)

    devices = jax.devices()[:NCORES]
    mesh = Mesh(np.asarray(devices), ("core",))
    sharded = jax.jit(
        shard_map(_body, mesh=mesh,
                  in_specs=(PartitionSpec("core"),) * (n_params + n_outs),
                  out_specs=(PartitionSpec("core"),) * n_outs,
                  check_rep=False),
        donate_argnums=donate, keep_unused=True)

    param_order = in_names[:n_params]

    def run(in_maps):
        concat_in = [
            np.concatenate([np.asarray(in_maps[c][nm]) for c in range(NCORES)],
                           axis=0)
            for nm in param_order
        ]
        concat_zeros = [np.zeros((NCORES * z.shape[0], *z.shape[1:]), z.dtype)
                        for z in zero_outs]
        out_arrs = jax.block_until_ready(sharded(*concat_in, *concat_zeros))
        return [
            {nm: np.asarray(out_arrs[i]).reshape(NCORES, *out_avals[i].shape)[c]
             for i, nm in enumerate(out_names)}
            for c in range(NCORES)
        ]

    return {"run": run, "sharded": sharded, "param_order": param_order,
            "zero_outs": zero_outs, "out_names": out_names,
            "out_avals": out_avals, "mesh": mesh}


def _get_runner(w, reps=None):
    import os
    mm = os.environ.get("KERNEL_MM_DTYPE", "bf16")
    if reps is None:
        reps = int(os.environ.get("KERNEL_REPS", "1"))
    phase = os.environ.get("KERNEL_PHASE", "all")
    key = (w, mm, reps, phase,
           os.environ.get("KERNEL_DVE_BD", ""),
           os.environ.get("KERNEL_POOL_N", ""),
           os.environ.get("KERNEL_POOL_BD", ""))
    if key not in _prog_cache:
        nc, WROW = _build_program(w, mm=mm, reps=reps, phase=phase)
        _prog_cache[key] = (_make_runner(nc), WROW)
    return _prog_cache[key]


def kernel(x4, attention, W1, b1, W2, b2):
    import ml_dtypes
    bf16 = ml_dtypes.bfloat16
    Xp, attp, perms, w = _prepare(x4, attention)
    runner, WROW = _get_runner(w, reps=1)

    Xpb = Xp.astype(bf16)
    xpt = np.zeros((B, F, WROW), bf16)
    xpt[:, :, :N] = Xpb.transpose(0, 2, 1)
    attp_pad = np.full((B, WROW), 1e9, np.float32)
    attp_pad[:, :N] = attp

    W1b = np.ascontiguousarray(W1, dtype=np.float32).astype(bf16)
    W2b = np.ascontiguousarray(W2, dtype=np.float32).astype(bf16)
    b1 = np.ascontiguousarray(b1, dtype=np.float32)
    b2 = np.ascontiguousarray(b2, dtype=np.float32)

    c_ones = np.ones((128, 1), bf16)
    c_estep = np.zeros((FCH, 95), bf16)
    c_estep[:, 47] = 1.0
    c_zeros = np.zeros((128, WROW), bf16)

    in_maps = []
    for c in range(NCORES):
        sl = slice(c * SPB, (c + 1) * SPB)
        in_maps.append({
            "xp": np.ascontiguousarray(Xpb[sl]),
            "xpt": np.ascontiguousarray(xpt[sl]),
            "attp": np.ascontiguousarray(attp_pad[sl]),
            "w1": W1b, "b1": b1, "w2": W2b, "b2": b2,
            "c_ones": c_ones, "c_estep": c_estep, "c_zeros": c_zeros,
        })

    results = runner["run"](in_maps)
    globals()["last_in_maps"] = in_maps
    globals()["last_runner"] = runner

    inv = np.argsort(perms, axis=1)
    out = np.empty((B, N, F), np.float32)
    for c in range(NCORES):
        o = results[c]["outT"]  # [SPB, F, N]
        for s in range(SPB):
            bi = c * SPB + s
            out[bi] = o[s].T[inv[bi]]
    return out


# revision 33
# speedup vs baseline: 1.3335x; 1.0620x over previous
"""Trainium2 Bass kernel: batched ChebConv GNN with L1-distance adjacency.

Pipeline per sample (N=512 nodes, F=625 features):
  1. Sort nodes by attention (host). All pairs with |att_i-att_j| <= 0.05
     then lie within a rank band |i-j| <= w (w computed exactly on host).
  2. Banded pairwise L1 distances on device via the exact identity
     sum_f |a-b| = 2*sum_f max(a,b) - S_i - S_j  (S = row sums). The max
     runs on DVE in bf16 (2x DVE throughput; masks flip on only 0.08% of
     band pairs vs fp32); the feature reduction is a PE staircase matmul
     into one PSUM row per band offset.
  3. Threshold masks -> banded adjacency [w, N] -> PE-transpose to
     [128, w] tiles -> skewed-contiguous DMA into a dense upper-triangle
     DRAM matrix (168B runs, not 4B diagonal elements). Lower triangle
     rebuilt in SBUF from the readback via PE transposes (A symmetric).
  4. deg via PE column-sum matmuls; reciprocal row broadcast; ChebConv x2
     as bf16 PE matmuls in transposed layouts.
Data parallel over batch: 16 samples, 8 cores, 2 samples/core, the two
samples unit-interleaved so DVE/Pool/PE/Act all stay fed.
"""

import numpy as np
from contextlib import ExitStack

B, N = 16, 512
F, FH = 625, 937
FCH, NFCH = 125, 5  # feature chunks: 5 x 125 = 625
NCORES = 8
SPB = B // NCORES  # samples per core
DIST_THRESH, ATT_THRESH = 180.0, 0.05
DCH = 48  # band offsets per PSUM group

# FH row blocks (7x128 + 41)
FH_BLOCKS = [(o, min(128, FH - o)) for o in range(0, FH, 128)]

_prog_cache = {}


def _build_program(w, mm="bf16", reps=1, phase="all"):
    """Build the SPMD Bass program for band half-width w. Returns (nc, WROW).

    phase: which section the reps hardware loop wraps ("all", "phase1",
    "band", "cheb", "empty") — ablation instrument; result stays correct.
    """
    import os as _os
    import concourse.bass as bass
    import concourse.bacc as bacc
    import concourse.mybir as mybir
    import concourse.tile as tile
    from concourse.masks import make_identity

    dt = mybir.dt
    fp = dt.float32
    bf = dt.bfloat16
    AF = mybir.ActivationFunctionType
    OP = mybir.AluOpType
    AP = bass.AP

    DVE_BD = int(_os.environ.get("KERNEL_DVE_BD", "6"))
    POOL_N = int(_os.environ.get("KERNEL_POOL_N", "0"))  # offsets/chunk on Pool
    POOL_BD = int(_os.environ.get("KERNEL_POOL_BD", "6"))

    padw = ((w + 7) // 8) * 8
    WROW = N + padw  # padded row width for xt rows / a_scr cols

    nc = bacc.Bacc()
    xp_p = nc.declare_dram_parameter("xp", [SPB, N, F], bf, isOutput=False)
    xpt_p = nc.declare_dram_parameter("xpt", [SPB, F, WROW], bf, isOutput=False)
    attp_p = nc.declare_dram_parameter("attp", [SPB, WROW], fp, isOutput=False)
    w1_p = nc.declare_dram_parameter("w1", [2, F, FH], bf, isOutput=False)
    b1_p = nc.declare_dram_parameter("b1", [FH], fp, isOutput=False)
    w2_p = nc.declare_dram_parameter("w2", [2, FH, F], bf, isOutput=False)
    b2_p = nc.declare_dram_parameter("b2", [F], fp, isOutput=False)
    out_p = nc.declare_dram_parameter("outT", [SPB, F, N], fp, isOutput=True)
    ones_p = nc.declare_dram_parameter("c_ones", [128, 1], bf, isOutput=False)
    estep_p = nc.declare_dram_parameter("c_estep", [FCH, 95], bf, isOutput=False)
    zeros_p = nc.declare_dram_parameter("c_zeros", [128, WROW], bf, isOutput=False)

    # internal DRAM scratch: dense adjacency rows, one per sample slot
    a_scr = [nc.dram_tensor(f"a_scr{b}", [WROW * WROW], bf) for b in range(SPB)]

    with tile.TileContext(nc) as tc, ExitStack() as ctx:
        cst = ctx.enter_context(tc.tile_pool(name="cst", bufs=1))
        xtp = ctx.enter_context(tc.tile_pool(name="xtp", bufs=2))
        xpp = ctx.enter_context(tc.tile_pool(name="xpp", bufs=1))
        mxp = ctx.enter_context(tc.tile_pool(name="mxp", bufs=4))
        mpp = ctx.enter_context(tc.tile_pool(name="mpp", bufs=3))
        bnd = ctx.enter_context(tc.tile_pool(name="bnd", bufs=2))
        amp = ctx.enter_context(tc.tile_pool(name="amp", bufs=1))
        acp = ctx.enter_context(tc.tile_pool(name="acp", bufs=1))
        wsp = ctx.enter_context(tc.tile_pool(name="wsp", bufs=1))
        otp = ctx.enter_context(tc.tile_pool(name="otp", bufs=1))
        psb = ctx.enter_context(tc.tile_pool(name="psb", bufs=1, space="PSUM"))
        pst = ctx.enter_context(tc.tile_pool(name="pst", bufs=2, space="PSUM"))
        psp = ctx.enter_context(tc.tile_pool(name="psp", bufs=2, space="PSUM"))

        ones = cst.tile([128, 1], bf, tag="ones")
        nc.scalar.dma_start(ones[:], ones_p[:, :])
        identb = cst.tile([128, 128], bf, tag="identb")
        make_identity(nc, identb[:])
        # staircase selector: estep[:, 47-di : 47-di+dn] is a [FCH, dn]
        # matrix whose only nonzero column is column di (all ones) -> matmul
        # with it as lhsT reduces partitions into PSUM row di
        estep = cst.tile([FCH, 95], bf, tag="estep")
        nc.scalar.dma_start(estep[:], estep_p[:, :])

        # ---- setup (once): resident weights/biases, a_scr zero init
        w1t = [[wsp.tile([FCH, FH], bf, tag=f"w1t{k_}{c_}", name=f"w1t{k_}{c_}")
                for c_ in range(NFCH)] for k_ in range(2)]
        for k_ in range(2):
            for c_ in range(NFCH):
                nc.scalar.dma_start(w1t[k_][c_][:],
                                    w1_p[k_, c_ * FCH:(c_ + 1) * FCH, :])
        w2t = [[wsp.tile([128, F], bf, tag=f"w2t{k_}{j_}", name=f"w2t{k_}{j_}")
                for j_ in range(len(FH_BLOCKS))] for k_ in range(2)]
        for k_ in range(2):
            for j_, (ko, kp) in enumerate(FH_BLOCKS):
                nc.scalar.dma_start(w2t[k_][j_][:kp, :],
                                    w2_p[k_, ko:ko + kp, :])
        b1t = [wsp.tile([128, 1], fp, tag=f"b1t{j_}", name=f"b1t{j_}")
               for j_ in range(len(FH_BLOCKS))]
        for j_, (mo, mp_) in enumerate(FH_BLOCKS):
            nc.scalar.dma_start(b1t[j_][:mp_, :], b1_p[mo:mo + mp_])
        b2t = [wsp.tile([FCH, 1], fp, tag=f"b2t{m_}", name=f"b2t{m_}")
               for m_ in range(NFCH)]
        for m_ in range(NFCH):
            nc.scalar.dma_start(b2t[m_][:], b2_p[m_ * FCH:(m_ + 1) * FCH])
        for b in range(SPB):
            ad = a_scr[b]
            for t in range(5):
                lo = t * 128 * WROW
                n_el = min(128 * WROW, WROW * WROW - lo)
                if n_el <= 0:
                    break
                nc.sync.dma_start(AP(ad, lo, [[1, n_el]]),
                                  AP(zeros_p, 0, [[1, n_el]]))

        rep_cm = tc.For_i(0, reps, 1) if reps > 1 else None
        _lo = [False]

        def _open():
            if rep_cm is not None and not _lo[0]:
                rep_cm.__enter__()
                _lo[0] = True

        def _close():
            if _lo[0]:
                rep_cm.__exit__(None, None, None)
                _lo[0] = False

        if phase in ("all", "phase1"):
            _open()

        # ---------------- per-sample state ----------------
        xt_all = [None] * SPB
        xn_all = [None] * SPB
        srow_all = [None] * SPB
        at_all = [None] * SPB
        dinvB_all = [None] * SPB
        psM_all = [None] * SPB  # [DCH, N] bank triple-duty: psS row, band M, deg

        def gen_phase1(b):
            # single 3D DMA fills all five feature chunks side by side
            xtb = xtp.tile([FCH, NFCH * WROW], bf, tag=f"xt{b}", name=f"xt{b}")
            a = xtb[:]
            s = xpt_p[b, :, :]
            nc.sync.dma_start(
                AP(a.tensor, a.offset,
                   [list(a.ap[0]), [WROW, NFCH], [1, WROW]]),
                AP(s.tensor, s.offset,
                   [[WROW, FCH], [FCH * WROW, NFCH], [1, WROW]]))
            xt = [xtb[:, c * WROW:(c + 1) * WROW] for c in range(NFCH)]
            xt_all[b] = xt
            # xn for cheb: one 3D DMA, issued early on SP queue
            xnb = xpp.tile([128, 4 * F], bf, tag=f"xn{b}", name=f"xn{b}")
            a = xnb[:]
            s = xp_p[b, :, :]
            nc.sync.dma_start(
                AP(a.tensor, a.offset, [list(a.ap[0]), [F, 4], [1, F]]),
                AP(s.tensor, s.offset, [[F, 128], [128 * F, 4], [1, F]]))
            xn = [xnb[:, t * F:(t + 1) * F] for t in range(4)]
            xn_all[b] = xn
            yield
            psMS = psb.tile([DCH, N], fp, tag=f"psM{b}", name=f"psM{b}")
            psM_all[b] = psMS
            for c in range(NFCH):
                nc.tensor.matmul(psMS[0:1, :], ones[:FCH, :], xt[c][:, :N],
                                 start=(c == 0), stop=(c == NFCH - 1))
            srow = bnd.tile([1, WROW], fp, tag=f"srow{b}", name=f"srow{b}")
            nc.gpsimd.memset(srow[:, N:], 0.0)
            nc.scalar.copy(srow[:, :N], psMS[0:1, :])
            srow_all[b] = srow
            yield

        def gen_band(b):
            ad = a_scr[b]
            xt, srow = xt_all[b], srow_all[b]
            psM = psM_all[b]
            d0 = 1
            while d0 <= w:
                dn = min(DCH, w - d0 + 1)
                # offsets [d0, d0+dn): tail POOL_N of them on Pool engine.
                # steps = (engine, offset-batch, chunk); pool steps merged
                # evenly among dve steps so the in-order PSUM consumption
                # chain lets both engines produce concurrently
                n_pool = min(POOL_N, max(0, dn - 1)) if POOL_N > 0 else 0
                n_dve = dn - n_pool
                dsteps, psteps = [], []
                o = 0
                while o < n_dve:
                    nb = min(DVE_BD, n_dve - o)
                    for c in range(NFCH):
                        dsteps.append(("dve", o, nb, c))
                    o += nb
                while o < dn:
                    nb = min(POOL_BD, dn - o)
                    for c in range(NFCH):
                        psteps.append(("pool", o, nb, c))
                    o += nb
                steps = []
                nd, np_ = len(dsteps), len(psteps)
                di_, pi_ = 0, 0
                while di_ < nd or pi_ < np_:
                    # keep pool's consumed fraction slightly ahead
                    if pi_ < np_ and (di_ >= nd or
                                      pi_ * nd <= di_ * np_):
                        steps.append(psteps[pi_])
                        pi_ += 1
                    else:
                        steps.append(dsteps[di_])
                        di_ += 1
                total_mm = dn * NFCH
                mm_done = 0
                for eng, db0, nb, c in steps:
                    tp = mxp if eng == "dve" else mpp
                    bd_ = DVE_BD if eng == "dve" else POOL_BD
                    mxb = tp.tile([FCH, bd_ * N], bf, tag=f"mx_{eng}",
                                  name=f"mx_{eng}")
                    # offset d only pairs i < N-d; clip batch to its widest
                    # offset (garbage beyond is att-masked in the epilogue)
                    wb = N - (d0 + db0)
                    base = xt[c][:, 0:N]
                    in0 = AP(base.tensor, base.offset,
                             [list(base.ap[0]), [0, nb], [1, wb]])
                    in1 = AP(base.tensor, base.offset + d0 + db0,
                             [list(base.ap[0]), [1, nb], [1, wb]])
                    e = nc.vector if eng == "dve" else nc.gpsimd
                    mv = mxb[:]
                    mxv = AP(mv.tensor, mv.offset,
                             [list(mv.ap[0]), [N, nb], [1, wb]])
                    e.tensor_tensor(out=mxv, in0=in0, in1=in1, op=OP.max)
                    for j in range(nb):
                        di = db0 + j
                        mm_done += 1
                        nc.tensor.matmul(
                            psM[:dn, :wb],
                            estep[:, 47 - di:47 - di + dn],
                            mxb[:, j * N:j * N + wb],
                            start=(mm_done == 1),
                            stop=(mm_done == total_mm))
                    yield
                # epilogue: D = 2M - S_i - S_{i+d}; masks -> abnd
                sv = srow[:, :]
                sshift = bnd.tile([dn, N], fp, tag=f"sshift{b}", bufs=1,
                                  name="sshift")
                nc.sync.dma_start(
                    sshift[:],
                    AP(sv.tensor, sv.offset + d0,
                       [list(sv.ap[0]), [1, dn], [1, N]]))
                sb_t = bnd.tile([dn, N], fp, tag=f"sb{b}", bufs=1, name="sb_t")
                nc.sync.dma_start(
                    sb_t[:],
                    AP(sv.tensor, sv.offset,
                       [list(sv.ap[0]), [0, dn], [1, N]]))
                ashift = bnd.tile([dn, N], fp, tag=f"ashift{b}", bufs=1,
                                  name="ashift")
                nc.scalar.dma_start(ashift[:],
                                    AP(attp_p, b * WROW + d0, [[1, dn], [1, N]]))
                ab_t = bnd.tile([dn, N], fp, tag=f"ab{b}", bufs=1, name="ab_t")
                nc.scalar.dma_start(ab_t[:],
                                    AP(attp_p, b * WROW, [[0, dn], [1, N]]))
                yield
                nc.vector.scalar_tensor_tensor(
                    out=sb_t[:], in0=sb_t[:], scalar=DIST_THRESH, in1=sshift[:],
                    op0=OP.add, op1=OP.add)
                nc.vector.scalar_tensor_tensor(
                    out=sshift[:], in0=psM[:dn, :], scalar=2.0, in1=sb_t[:],
                    op0=OP.mult, op1=OP.is_le)
                nc.vector.tensor_sub(ashift[:], ashift[:], ab_t[:])
                nc.vector.tensor_scalar(ab_t[:], ashift[:], ATT_THRESH, None,
                                        op0=OP.is_le)
                abnd = bnd.tile([dn, N], bf, tag=f"abnd{b}", name="abnd")
                nc.vector.tensor_mul(abnd[:], sshift[:], ab_t[:])
                yield
                # transpose to [128, dn] blocks; skewed-contiguous scatter of
                # the upper triangle: A[i, i+d] for d in [d0, d0+dn) lands at
                # a_scr[i*(WROW+1) + d0 + f], contiguous runs of dn elems
                abT = bnd.tile([128, 4 * DCH], bf, tag=f"abT{b}",
                               name="abT", bufs=2)
                for t in range(4):
                    psTa = pst.tile([128, 128], bf, tag="pst",
                                    name="psTa")
                    nc.tensor.transpose(psTa[:, :dn],
                                        abnd[:, t * 128:(t + 1) * 128],
                                        identb[:dn, :dn])
                    nc.scalar.copy(abT[:, t * DCH:t * DCH + dn],
                                   psTa[:, :dn])
                    yield
                av = abT[:]
                nc.sync.dma_start(
                    AP(ad, d0, [[WROW + 1, 128],
                                [128 * (WROW + 1), 4], [1, dn]]),
                    AP(av.tensor, av.offset,
                       [list(av.ap[0]), [DCH, 4], [1, dn]]))
                yield
                d0 += dn

            # dense readback (upper + zeros elsewhere), rebuild lower
            # triangle from symmetric upper via PE transposes
            atb = amp.tile([128, 4 * N], bf, tag=f"at{b}", name=f"at{b}")
            a = atb[:]
            nc.sync.dma_start(
                AP(a.tensor, a.offset, [list(a.ap[0]), [N, 4], [1, N]]),
                AP(ad, 0, [[WROW, 128], [128 * WROW, 4], [1, N]]))
            at = [atb[:, t * N:(t + 1) * N] for t in range(4)]
            yield
            for t in range(4):
                # diagonal block: lower part = upper(t,t)^T; also add I
                psT2 = pst.tile([128, 128], bf, tag="pst", name="psT2")
                nc.tensor.transpose(psT2[:],
                                    at[t][:, t * 128:(t + 1) * 128],
                                    identb[:])
                nc.vector.tensor_tensor(
                    out=at[t][:, t * 128:(t + 1) * 128],
                    in0=at[t][:, t * 128:(t + 1) * 128],
                    in1=psT2[:], op=OP.add)
                nc.vector.tensor_tensor(
                    out=at[t][:, t * 128:(t + 1) * 128],
                    in0=at[t][:, t * 128:(t + 1) * 128],
                    in1=identb[:], op=OP.add)
                if t > 0:
                    # wedge: rows of block t, cols of block t-1
                    psT3 = pst.tile([128, 128], bf, tag="pst",
                                    name="psT3")
                    nc.tensor.transpose(psT3[:],
                                        at[t - 1][:, t * 128:(t + 1) * 128],
                                        identb[:])
                    nc.vector.tensor_tensor(
                        out=at[t][:, (t - 1) * 128:t * 128],
                        in0=at[t][:, (t - 1) * 128:t * 128],
                        in1=psT3[:], op=OP.add)
                yield
            # deg[j] = colsum (A+I); scale cols by 1/deg
            psM2 = psM_all[b]
            for t in range(4):
                nc.tensor.matmul(psM2[0:1, :], ones[:, :], at[t][:],
                                 start=(t == 0), stop=(t == 3))
            dinvR = bnd.tile([1, N], fp, tag=f"dinvR{b}", name=f"dinvR{b}")
            nc.vector.reciprocal(dinvR[:], psM2[0:1, :])
            dinvB = amp.tile([128, N], fp, tag=f"dinvB{b}", name=f"dinvB{b}")
            nc.gpsimd.partition_broadcast(dinvB[:], dinvR[:, :])
            dinvB_all[b] = dinvB
            at_all[b] = at
            yield

        def gen_cheb(b):
            xt, at, xn = xt_all[b], at_all[b], xn_all[b]
            dinvB = dinvB_all[b]
            zt = [acp.tile([FCH, N], bf, tag=f"zt{b}{m}", name=f"zt{b}{m}")
                  for m in range(NFCH)]
            for m in range(NFCH):
                psZ = psp.tile([FCH, N], fp, tag=f"mm{b}", name="psZ")
                for t in range(4):
                    nc.tensor.matmul(psZ[:], xn[t][:, m * FCH:(m + 1) * FCH],
                                     at[t][:], start=(t == 0), stop=(t == 3))
                nc.vector.tensor_mul(zt[m][:], psZ[:], dinvB[:FCH, :])
                yield

            ht = [acp.tile([128, N], bf, tag=f"ht{b}{k}", name=f"ht{b}{k}")
                  for k in range(len(FH_BLOCKS))]
            for k, (mo, mp_) in enumerate(FH_BLOCKS):
                psH = psp.tile([128, N], fp, tag=f"mm{b}", name="psH")
                for c in range(NFCH):
                    nc.tensor.matmul(psH[:mp_, :], w1t[0][c][:, mo:mo + mp_],
                                     xt[c][:, :N], start=(c == 0), stop=False)
                for c in range(NFCH):
                    nc.tensor.matmul(psH[:mp_, :], w1t[1][c][:, mo:mo + mp_],
                                     zt[c][:], start=False,
                                     stop=(c == NFCH - 1))
                nc.scalar.activation(ht[k][:mp_, :], psH[:mp_, :], AF.Relu,
                                     bias=b1t[k][:mp_, :], scale=1.0)
                yield

            qt = [acp.tile([128, N], bf, tag=f"qt{b}{k}", name=f"qt{b}{k}")
                  for k in range(len(FH_BLOCKS))]
            for k, (mo, mp_) in enumerate(FH_BLOCKS):
                psQ = psp.tile([128, N], fp, tag=f"mm{b}", name="psQ")
                for t in range(4):
                    psT = pst.tile([128, 128], bf, tag="pst", name="psT")
                    nc.tensor.transpose(
                        psT[:, :mp_],
                        ht[k][:mp_, t * 128:(t + 1) * 128],
                        identb[:mp_, :mp_])
                    hb = bnd.tile([128, 128], bf, tag=f"hb{b}", bufs=3,
                                  name="hb")
                    nc.scalar.copy(hb[:, :mp_], psT[:, :mp_])
                    nc.tensor.matmul(psQ[:mp_, :], hb[:, :mp_], at[t][:],
                                     start=(t == 0), stop=(t == 3))
                nc.vector.tensor_mul(qt[k][:mp_, :], psQ[:mp_, :],
                                     dinvB[:mp_, :])
                yield

            for m in range(NFCH):
                psO = psp.tile([FCH, N], fp, tag=f"mm{b}", name="psO")
                for k, (ko, kp) in enumerate(FH_BLOCKS):
                    nc.tensor.matmul(psO[:],
                                     w2t[0][k][:kp, m * FCH:(m + 1) * FCH],
                                     ht[k][:kp, :], start=(k == 0), stop=False)
                for k, (ko, kp) in enumerate(FH_BLOCKS):
                    nc.tensor.matmul(psO[:],
                                     w2t[1][k][:kp, m * FCH:(m + 1) * FCH],
                                     qt[k][:kp, :], start=False,
                                     stop=(k == len(FH_BLOCKS) - 1))
                ot = otp.tile([FCH, N], fp, tag=f"ot{b}", name="ot")
                nc.scalar.activation(ot[:], psO[:], AF.Relu, bias=b2t[m][:],
                                     scale=1.0)
                nc.gpsimd.dma_start(out_p[b, m * FCH:(m + 1) * FCH, :], ot[:])
                yield

        def rr(*gens):
            gens = list(gens)
            while gens:
                g = gens.pop(0)
                if next(g, StopIteration) is not StopIteration:
                    gens.append(g)

        def stagger(g_a, g_b, ratio=2):
            # drive g_a `ratio` steps per g_b step until both exhausted
            done_a = done_b = False
            while not (done_a and done_b):
                for _ in range(ratio):
                    if not done_a:
                        done_a = next(g_a, StopIteration) is StopIteration
                if not done_b:
                    done_b = next(g_b, StopIteration) is StopIteration

        if phase == "all":
            rr(gen_phase1(0), gen_phase1(1))
            for _ in gen_band(0):
                pass
            stagger(gen_band(1), gen_cheb(0), ratio=2)
            for _ in gen_cheb(1):
                pass
        else:
            rr(gen_phase1(0), gen_phase1(1))
            if phase == "phase1":
                _close()
            if phase == "band":
                _open()
            rr(gen_band(0), gen_band(1))
            if phase == "band":
                _close()
            if phase == "cheb":
                _open()
            rr(gen_cheb(0), gen_cheb(1))
            if phase == "cheb":
                _close()
            if phase == "empty":
                _open()
                dummy = bnd.tile([1, 1], fp, tag="dummy", name="dummy")
                nc.scalar.copy(dummy[:], ones[:1, :].bitcast(fp))
                _close()
        _close()

    if not nc.is_finalized():
        nc.finalize()
    return nc, WROW


def _prepare(x4, attention):
    """Host prep: flatten, sort by attention, compute band width, pad."""
    X = np.ascontiguousarray(x4.reshape(B, N, F), dtype=np.float32)
    att = np.ascontiguousarray(attention[:, :, 0, 0], dtype=np.float32)
    perms = np.argsort(att, axis=1, kind="stable")
    attp = np.take_along_axis(att, perms, axis=1)
    a64 = attp.astype(np.float64)
    w = 1
    for bi in range(B):
        for d in range(1, N):
            if np.min(a64[bi, d:] - a64[bi, :-d]) <= ATT_THRESH + 1e-6:
                w = max(w, d)
            else:
                break  # windows only widen with d
    w = min(w, N - 1)
    Xp = np.take_along_axis(X, perms[:, :, None], axis=1)
    return Xp, attp, perms, w


def _make_runner(nc):
    """Compile the Bass program into a reusable 8-core sharded jax callable.

    Mirrors concourse.bass2jax.run_bass_via_pjrt's multi-core branch, but
    returns the compiled callable so repeated executions can be timed.
    """
    import jax
    from jax.sharding import Mesh, PartitionSpec
    from jax.experimental.shard_map import shard_map
    from concourse import bass2jax, mybir

    bass2jax.install_neuronx_cc_hook()

    in_names, out_names, out_avals, zero_outs = [], [], [], []
    partition_name = (nc.partition_id_tensor.name
                      if nc.partition_id_tensor else None)
    for alloc in nc.m.functions[0].allocations:
        if not isinstance(alloc, mybir.MemoryLocationSet):
            continue
        name = alloc.memorylocations[0].name
        if alloc.kind == "ExternalInput":
            if name != partition_name:
                in_names.append(name)
        elif alloc.kind == "ExternalOutput":
            shape = tuple(alloc.tensor_shape)
            dtype = mybir.dt.np(alloc.dtype)
            out_names.append(name)
            out_avals.append(jax.core.ShapedArray(shape, dtype))
            zero_outs.append(np.zeros(shape, dtype))
    n_params = len(in_names)
    n_outs = len(out_avals)
    in_names = in_names + out_names
    if partition_name is not None:
        in_names.append(partition_name)
    donate = tuple(range(n_params, n_params + n_outs))

    def _body(*args):
        operands = list(args)
        if partition_name is not None:
            operands.append(bass2jax.partition_id_tensor())
        outs = bass2jax._bass_exec_p.bind(
            *operands,
            out_avals=tuple(out_avals),
            in_names=tuple(in_names),
            out_names=tuple(out_names),
            lowering_input_output_aliases=(),
            sim_require_finite=True,
            sim_require_nnan=True,
            nc=nc,
        )
        return tuple(outs)

    devices = jax.devices()[:NCORES]
    mesh = Mesh(np.asarray(devices), ("core",))
    sharded = jax.jit(
        shard_map(_body, mesh=mesh,
                  in_specs=(PartitionSpec("core"),) * (n_params + n_outs),
                  out_specs=(PartitionSpec("core"),) * n_outs,
                  check_rep=False),
        donate_argnums=donate, keep_unused=True)

    param_order = in_names[:n_params]

    def run(in_maps):
        concat_in = [
            np.concatenate([np.asarray(in_maps[c][nm]) for c in range(NCORES)],
                           axis=0)
            for nm in param_order
        ]
        concat_zeros = [np.zeros((NCORES * z.shape[0], *z.shape[1:]), z.dtype)
                        for z in zero_outs]
        out_arrs = jax.block_until_ready(sharded(*concat_in, *concat_zeros))
        return [
            {nm: np.asarray(out_arrs[i]).reshape(NCORES, *out_avals[i].shape)[c]
             for i, nm in enumerate(out_names)}
            for c in range(NCORES)
        ]

    return {"run": run, "sharded": sharded, "param_order": param_order,
            "zero_outs": zero_outs, "out_names": out_names,
            "out_avals": out_avals, "mesh": mesh}


def _get_runner(w, reps=None):
    import os
    mm = os.environ.get("KERNEL_MM_DTYPE", "bf16")
    if reps is None:
        reps = int(os.environ.get("KERNEL_REPS", "1"))
    phase = os.environ.get("KERNEL_PHASE", "all")
    key = (w, mm, reps, phase,
           os.environ.get("KERNEL_DVE_BD", ""),
           os.environ.get("KERNEL_POOL_N", ""),
           os.environ.get("KERNEL_POOL_BD", ""))
    if key not in _prog_cache:
        nc, WROW = _build_program(w, mm=mm, reps=reps, phase=phase)
        _prog_cache[key] = (_make_runner(nc), WROW)
    return _prog_cache[key]


def kernel(x4, attention, W1, b1, W2, b2):
    import ml_dtypes
    bf16 = ml_dtypes.bfloat16
    Xp, attp, perms, w = _prepare(x4, attention)
    runner, WROW = _get_runner(w, reps=1)

    Xpb = Xp.astype(bf16)
    xpt = np.zeros((B, F, WROW), bf16)
    xpt[:, :, :N] = Xpb.transpose(0, 2, 1)
    attp_pad = np.full((B, WROW), 1e9, np.float32)
    attp_pad[:, :N] = attp

    W1b = np.ascontiguousarray(W1, dtype=np.float32).astype(bf16)
    W2b = np.ascontiguousarray(W2, dtype=np.float32).astype(bf16)
    b1 = np.ascontiguousarray(b1, dtype=np.float32)
    b2 = np.ascontiguousarray(b2, dtype=np.float32)

    c_ones = np.ones((128, 1), bf16)
    c_estep = np.zeros((FCH, 95), bf16)
    c_estep[:, 47] = 1.0
    c_zeros = np.zeros((128, WROW), bf16)

    in_maps = []
    for c in range(NCORES):
        sl = slice(c * SPB, (c + 1) * SPB)
        in_maps.append({
            "xp": np.ascontiguousarray(Xpb[sl]),
            "xpt": np.ascontiguousarray(xpt[sl]),
            "attp": np.ascontiguousarray(attp_pad[sl]),
            "w1": W1b, "b1": b1, "w2": W2b, "b2": b2,
            "c_ones": c_ones, "c_estep": c_estep, "c_zeros": c_zeros,
        })

    results = runner["run"](in_maps)
    globals()["last_in_maps"] = in_maps
    globals()["last_runner"] = runner

    inv = np.argsort(perms, axis=1)
    out = np.empty((B, N, F), np.float32)
    for c in range(NCORES):
        o = results[c]["outT"]  # [SPB, F, N]
        for s in range(SPB):
            bi = c * SPB + s
            out[bi] = o[s].T[inv[bi]]
    return out
